# revision 21
# baseline (speedup 1.0000x reference)
"""Trainium2 Bass kernel for nn_FCGF_point_att3_sft_7000 (8 NeuronCores).

Model: pointwise attention MLP (32->16->8->1, BN+relu, BN stats over the full
512000-point batch), per-segment softmax over 2000 points, attention-weighted
pooling to [256, 64000], FC head 64000->1024->256 (BN+relu, stats over the
256-segment batch), final L2 row-normalize.

Sharding: points-within-segment. Core c owns points p in [250c, 250(c+1)) of
every segment. Stage A is data-parallel over points with AllGather'd BN stats;
fc1 is contraction-sharded (each core owns 8000 of the 64000 inputs and the
matching fw1 rows), summed via ReduceScatter whose per-shard aux row also
carries the softmax denominators; fc2 is contraction-sharded and finished with
an AllReduce; the tail is replicated.

Layout notes:
- Stage A layers 1-2 run in "quartered" A-orientation: x.T is [128, 16000]
  with the channels of free-quarter a on partitions [32a, 32a+32), matmuls
  use a block-diagonal lhsT so every chunk op runs 128 partitions wide.
- Layer 3 uses transpose-matmuls: lhsT = h2 point-chunk [128, 125], rhs =
  w3 spread over the 4 quarter blocks [128, 4] -> scores land point-major in
  a single PSUM bank [125, (seg64, half2, quarter4)].  relu/exp/softmax-z all
  run on that tiny tile; the old y3 eviction / repack / PE-transposes vanish.
- A zero-dependency warmup AllGather triggers at t~0 so the ~50us ncfw init
  overlaps the input DMAs and stage-A layer 1.
- fw1 streams as two big contiguous DMAs (96KB + 32KB per partition runs).
- ReduceScatter runs in fp16 (partials + softmax z), AllReduce in f32.

Training-mode BN is shift-invariant => conv/linear biases (b1,b2,b3,fb1,fb2)
drop out exactly; they are accepted and ignored.
"""

import sys

sys.path.insert(0, "/opt/trn_rl_repo")

import numpy as np

import concourse.bass as bass
import concourse.tile as tile
from concourse import mybir
from concourse.masks import make_identity

B = 256
P = 2000
C = 32
NCORES = 8
PL = P // NCORES           # 250
PH = PL // 2               # 125
NPTS = B * PL              # 64000 points per core
QF = NPTS // 4             # 16000 per quarter
NCH = 1000                 # stage-A eviction chunk (two 500-col matmuls)
NCHUNK = QF // NCH         # 16
FW_SPLIT = 44              # fc1 its in the first (big) weight DMA
EPS_BN = 1e-5
F32 = mybir.dt.float32
BF16 = mybir.dt.float16  # fp16: same speed as bf16, 8x lower rounding noise
RG = [list(range(NCORES))]
AF = mybir.ActivationFunctionType
ALU = mybir.AluOpType

_cache = {}


# ------------------------------------------------------------------ walrus fix
def _install_walrus_patch():
    """This container's walrus accepts only ONE semaphore wait per instruction.
    Spread Tile's end-of-kernel drain waits across single-wait nops, and split
    any instruction carrying >1 waits onto same-engine carrier nops."""
    if _cache.get("patched"):
        return
    from concourse.vector_clock import ScopedClock, VectorClock

    counter = [0]

    def split_waits(nc):
        for bb in nc.main_func.blocks:
            out = []
            changed = False
            for ins in bb.instructions:
                si = ins.sync_info
                waits = list(si.on_wait) if si and si.on_wait else []
                if len(waits) > 1:
                    changed = True
                    for w in waits[:-1]:
                        counter[0] += 1
                        out.append(mybir.InstNoOp(
                            name=f"I-wsplit-{counter[0]}",
                            engine=ins.engine, ins=[], outs=[],
                            sync_info=mybir.SyncInfo(on_wait=[w], on_update=[]),
                            bass_nofuse=True))
                    si.on_wait = waits[-1:]
                out.append(ins)
            if changed:
                try:
                    bb.instructions = out
                except Exception:
                    bb.instructions.clear()
                    for x in out:
                        bb.instructions.append(x)

    def _patched(self, tick_clock, wait_clock):
        nc = self.nc
        gc = tick_clock.global_clock
        n = len(gc)
        for i in range(n):
            if gc[i] > 0:
                vec = [0] * n
                vec[i] = gc[i]
                nop = nc.sync.nop(nofuse=True, hint=f"drain_wait_p{i}")
                wait_clock.add_sem_waits(
                    nop.ins, ScopedClock({None: VectorClock(vec)}))
        nc.sync.drain()
        nc.all_engine_barrier()
        assert self.sems is not None
        popped = nc._tile_sem_poison_stack.pop()
        assert popped is self._sem_poison
        nc.clear_and_free_semaphores(list(self.sems.allocated().values()))
        nc.all_engine_barrier()
        split_waits(nc)

    tile.TileContext._drain_and_barrier = _patched
    _cache["patched"] = True


# ------------------------------------------------------------------ bass build
def _build():
    _install_walrus_patch()
    nc = bass.Bass()

    def ein(name, shape, dt):
        return nc.dram_tensor(name, shape, dt, kind="ExternalInput")

    d = {}
    d["xA4"] = ein("xA4", [128, QF], BF16)
    d["xB"] = ein("xB", [PH, C * 2 * B], BF16)
    d["w1D"] = ein("w1D", [128, 128], BF16)
    d["w2D"] = ein("w2D", [128, 128], BF16)
    d["w3sp"] = ein("w3sp", [128, 4], BF16)
    for n in ("g1q", "be1q", "g2q", "be2q"):
        d[n] = ein(n, [128, 1], F32)
    d["g3s"] = ein("g3s", [1, 1], F32)
    d["be3s"] = ein("be3s", [1, 1], F32)
    d["f1"] = ein("f1", [128, 16], F32)
    d["ft1"] = ein("ft1", [16, 128], F32)
    d["f2"] = ein("f2", [128, 8], F32)
    d["ft2"] = ein("ft2", [8, 128], F32)
    d["f8_16"] = ein("f8_16", [128, 16], F32)
    d["f8_8"] = ein("f8_8", [64, 8], F32)
    d["fw1t"] = ein("fw1t", [PH, C * 2 * 1024], BF16)
    d["fw2t"] = ein("fw2t", [128, 256], BF16)
    d["fg1s"] = ein("fg1s", [128, 1], F32)
    d["fbe1s"] = ein("fbe1s", [128, 1], F32)
    d["fg2t"] = ein("fg2t", [128, 2], F32)
    d["fbe2t"] = ein("fbe2t", [128, 2], F32)
    d["out_final"] = nc.dram_tensor("out_final", [256, 256], F32,
                                    kind="ExternalOutput")
    # collective bounce buffers
    d["warm_i"] = nc.dram_tensor("warm_i", [1, 4], F32)
    d["warm_o"] = nc.dram_tensor("warm_o", [8, 4], F32)
    d["st1_i"] = nc.dram_tensor("st1_i", [16, 2], F32)
    d["st1_o"] = nc.dram_tensor("st1_o", [128, 2], F32)
    d["st2_i"] = nc.dram_tensor("st2_i", [8, 2], F32)
    d["st2_o"] = nc.dram_tensor("st2_o", [64, 2], F32)
    d["st3_i"] = nc.dram_tensor("st3_i", [1, 2], F32)
    d["st3_o"] = nc.dram_tensor("st3_o", [8, 2], F32)
    d["rs5_i"] = nc.dram_tensor("rs5_i", [NCORES * 129, 256], BF16)
    d["rs5_o"] = nc.dram_tensor("rs5_o", [129, 256], BF16)
    d["ar6_i"] = nc.dram_tensor("ar6_i", [256, 256], F32)
    d["ar6_o"] = nc.dram_tensor("ar6_o", [256, 256], F32)

    with tile.TileContext(nc) as tc:
        _body(nc, tc, d)
    return nc


def _mkstats(nc, pool, mv, count, name):
    """mv [p,2]=(mean,var) -> (sum,sumsq) [p,2]."""
    p = mv.shape[0]
    ss = pool.tile([p, 2], F32, tag=f"ss_{name}")
    nc.vector.tensor_mul(ss[:, 1:2], mv[:, 0:1], mv[:, 0:1])
    nc.vector.tensor_add(ss[:, 1:2], ss[:, 1:2], mv[:, 1:2])
    nc.scalar.mul(ss[:, 0:1], mv[:, 0:1], float(count))
    nc.scalar.mul(ss[:, 1:2], ss[:, 1:2], float(count))
    return ss


def _mv_from_ss(nc, pool, ss, count, name):
    """(sum,sumsq) [p,2] over count -> (mean, rstd) [p,2]."""
    p = ss.shape[0]
    mr = pool.tile([p, 2], F32, tag=f"mr_{name}")
    epst = pool.tile([p, 1], F32, tag=f"eps_{name}")
    nc.vector.memset(epst[:], EPS_BN)
    nc.scalar.mul(mr[:, 0:1], ss[:, 0:1], 1.0 / count)
    nc.scalar.mul(mr[:, 1:2], ss[:, 1:2], 1.0 / count)
    m2 = pool.tile([p, 1], F32, tag=f"m2_{name}")
    nc.vector.tensor_mul(m2[:], mr[:, 0:1], mr[:, 0:1])
    nc.vector.tensor_sub(mr[:, 1:2], mr[:, 1:2], m2[:])
    nc.scalar.activation(mr[:, 1:2], mr[:, 1:2], AF.Sqrt, bias=epst[:])
    nc.vector.reciprocal(mr[:, 1:2], mr[:, 1:2])
    return mr


def _scale_bias(nc, pool, mrq, g, be, name):
    """scale = g*rstd ; bias = be - scale*mean  (all [p,1] per-partition)."""
    p = mrq.shape[0]
    sc = pool.tile([p, 1], F32, tag=f"sc_{name}")
    bi = pool.tile([p, 1], F32, tag=f"bi_{name}")
    nc.vector.tensor_mul(sc[:], g[:], mrq[:, 1:2])
    nc.vector.tensor_mul(bi[:], sc[:], mrq[:, 0:1])
    nc.vector.tensor_sub(bi[:], be[:], bi[:])
    return sc, bi


def _body(nc, tc, d):
    # Warmup collective: zero data deps beyond a 4-float gpsimd memset+DMA, so
    # the trigger fires ~t=0 and the ~50us ncfw init overlaps the whole front.
    warm_cm = tc.tile_pool(name="warm", bufs=1)
    warm = warm_cm.__enter__()
    wsb = warm.tile([1, 4], F32, tag="wsb")
    nc.gpsimd.memset(wsb[:], 0.0)
    nc.gpsimd.dma_start(d["warm_i"][:], wsb[:])
    nc.gpsimd.collective_compute(
        "AllGather", ALU.bypass, replica_groups=RG,
        ins=[d["warm_i"][:]], outs=[d["warm_o"][:]])

    sing_cm = tc.tile_pool(name="sing", bufs=1)
    bigY_cm = tc.tile_pool(name="bigY", bufs=1)   # xb + exp tiles: live to FC1
    work_cm = tc.tile_pool(name="work", bufs=1)
    fwA_cm = tc.tile_pool(name="fwA", bufs=1)
    bigX_cm = tc.tile_pool(name="bigX", bufs=1)   # xa/h2 + h1: dies after L3
    psA_cm = tc.tile_pool(name="psA", bufs=2, space="PSUM")
    psS_cm = tc.tile_pool(name="psS", bufs=1, space="PSUM")
    sing = sing_cm.__enter__()
    bigY = bigY_cm.__enter__()
    work = work_cm.__enter__()
    fwA_p = fwA_cm.__enter__()
    bigX = bigX_cm.__enter__()
    psA = psA_cm.__enter__(); psS = psS_cm.__enter__()

    # ---------------- constants (sync ring; small)
    def load(name, shape, dt=F32, pool=sing):
        t = pool.tile(shape, dt, tag=name)
        nc.sync.dma_start(t[:], d[name][:])
        return t

    w1D = load("w1D", [128, 128], BF16)
    w2D = load("w2D", [128, 128], BF16)
    w3sp = load("w3sp", [128, 4], BF16)
    f1s = load("f1", [128, 16])
    ft1s = load("ft1", [16, 128])
    f2s = load("f2", [128, 8])
    ft2s = load("ft2", [8, 128])
    f8_16s = load("f8_16", [128, 16])
    f8_8s = load("f8_8", [64, 8])
    g1 = load("g1q", [128, 1]); be1 = load("be1q", [128, 1])
    g2 = load("g2q", [128, 1]); be2 = load("be2q", [128, 1])
    g3 = load("g3s", [1, 1]); be3 = load("be3s", [1, 1])
    ones128 = sing.tile([128, 1], F32)
    nc.vector.memset(ones128[:], 1.0)
    ones8 = sing.tile([8, 1], F32)
    nc.vector.memset(ones8[:], 1.0)
    ones125 = sing.tile([PH, 1], F32)
    nc.vector.memset(ones125[:], 1.0)
    ones1x = sing.tile([1, 128], F32)
    nc.vector.memset(ones1x[:], 1.0)
    ident = sing.tile([128, 128], F32)
    make_identity(nc, ident[:])

    # ---------------- big loads: one fat DMA each.  xa rides the sync ring
    # (needed first, for L1); the scalar ring does fw1-head then xb (xb is
    # only consumed by FC1, so it queues behind the big weight stream).
    xa = bigX.tile([128, QF], BF16, tag="slotA")      # xa -> (dead) -> h2
    nc.sync.dma_start(xa[:], d["xA4"][:])
    fwv = d["fw1t"][:].rearrange("p (i o) -> p i o", i=C * 2, o=1024)
    fwAt = fwA_p.tile([PH, FW_SPLIT, 1024], BF16, tag="fwA")
    nc.scalar.dma_start(fwAt[:], fwv[:, :FW_SPLIT, :])
    xb = bigY.tile([PH, C * 2 * B], BF16, tag="xb")
    nc.scalar.dma_start(xb[:], d["xB"][:])
    xbv = xb[:].rearrange("p (c h a s) -> p c h a s", c=C, h=2, a=4, s=64)

    def stage_layer(rhs_src, wD, fold, foldT, f8fold, st_i, st_o,
                    gq, beq, name, out_tag):
        """Quartered A-orientation layer: matmuls -> raw evict (scalar) +
        bn_stats (vector, from PSUM), fold + AllGather stats, then BN+relu
        applied in place, split scalar/vector."""
        y = bigX.tile([128, QF], BF16, tag=out_tag, name=f"y_{name}")
        stat = work.tile([128, 2 * NCHUNK, 6], F32, tag=f"stat_{name}")
        for j in range(NCHUNK):
            ps = psA.tile([128, 1024], F32, tag="psA", name=f"ps_{name}_{j}")
            base = j * NCH
            nc.tensor.matmul(ps[:, 0:500], wD[:], rhs_src[:, base:base + 500],
                             start=True, stop=True)
            nc.tensor.matmul(ps[:, 512:1012], wD[:],
                             rhs_src[:, base + 500:base + 1000],
                             start=True, stop=True)
            pv = ps[:].rearrange("p (k c) -> p k c", k=2, c=512)[:, :, 0:500]
            nc.scalar.copy(
                y[:, base:base + NCH].rearrange("p (k c) -> p k c", k=2,
                                                c=500), pv)
            nc.vector.bn_stats(stat[:, 2 * j, :], ps[:, 0:500])
            nc.vector.bn_stats(stat[:, 2 * j + 1, :], ps[:, 512:1012])
        mv = work.tile([128, 2], F32, tag=f"mv_{name}")
        nc.vector.bn_aggr(mv[:], stat[:])
        ss = _mkstats(nc, work, mv, QF, name)
        nfold = fold.shape[1]
        psf = psS.tile([128, 2], F32, tag="small", name=f"psf_{name}")
        nc.tensor.matmul(psf[:nfold, :], fold[:], ss[:], start=True, stop=True)
        sbf = work.tile([nfold, 2], F32, tag=f"sbf_{name}")
        nc.scalar.copy(sbf[:], psf[:nfold, :])
        nc.gpsimd.dma_start(st_i[:], sbf[:])
        nc.gpsimd.collective_compute(
            "AllGather", ALU.bypass, replica_groups=RG,
            ins=[st_i[:]], outs=[st_o[:]])
        agg = work.tile([nfold * NCORES, 2], F32, tag=f"agg_{name}")
        nc.gpsimd.dma_start(agg[:], st_o[:])
        psg = psS.tile([128, 2], F32, tag="small", name=f"psg_{name}")
        nc.tensor.matmul(psg[:nfold, :], f8fold[:], agg[:], start=True,
                         stop=True)
        ssg = work.tile([nfold, 2], F32, tag=f"ssg_{name}")
        nc.scalar.copy(ssg[:], psg[:nfold, :])
        mr = _mv_from_ss(nc, work, ssg, B * P, name)
        psb = psS.tile([128, 2], F32, tag="small", name=f"psb_{name}")
        nc.tensor.matmul(psb[:], foldT[:], mr[:], start=True, stop=True)
        mrq = work.tile([128, 2], F32, tag=f"mrq_{name}")
        nc.scalar.copy(mrq[:], psb[:])
        sc, bi = _scale_bias(nc, work, mrq, gq, beq, name)
        # relu in place: scalar takes the first chunks, vector the rest
        NSC = 6
        for j in range(NSC):
            sl = slice(j * NCH, (j + 1) * NCH)
            nc.scalar.activation(y[:, sl], y[:, sl], AF.Relu,
                                 bias=bi[:], scale=sc[:])
        for j in range(NSC, NCHUNK):
            sl = slice(j * NCH, (j + 1) * NCH)
            nc.vector.tensor_scalar(y[:, sl], y[:, sl], sc[:], bi[:],
                                    ALU.mult, ALU.add)
            nc.vector.tensor_scalar_max(y[:, sl], y[:, sl], 0.0)
        return y

    # ---------------- stage A layers 1 & 2
    h1 = stage_layer(xa, w1D, f1s, ft1s, f8_16s,
                     d["st1_i"], d["st1_o"], g1, be1, "l1", "slotB")
    # h2 reuses slot A (xa dead after L1 matmuls)
    h2 = stage_layer(h1, w2D, f2s, ft2s, f8_8s,
                     d["st2_i"], d["st2_o"], g2, be2, "l2", "slotA")

    # ---------------- layer 3 via transpose-matmuls: scores point-major.
    # lhsT = h2[:, 125c:125c+125] (K=128 channel-partitions, M=125 points),
    # rhs = w3 spread [128, 4] (col a = w3 in quarter-a rows) ->
    # psL3[:, 4c+a] = score of quarter a's point 125c+p.
    # Free-dim layout: c = (s, h) with s in 0..63, h in 0..1; col = 8s+4h+a.
    psL3_cm = tc.tile_pool(name="psL3", bufs=1, space="PSUM")
    psL3 = psL3_cm.__enter__()
    l3ps = psL3.tile([PH, 512], F32, tag="l3ps")
    for cgrp in range(128):
        nc.tensor.matmul(l3ps[:, 4 * cgrp: 4 * cgrp + 4],
                         h2[:, 125 * cgrp: 125 * cgrp + 125],
                         w3sp[:], start=True, stop=True)
    # BN3 stats over all points (125*512 = 64000 local)
    stat3 = work.tile([PH, 6], F32, tag="stat3")
    nc.vector.bn_stats(stat3[:], l3ps[:])
    mv3 = work.tile([PH, 2], F32, tag="mv3")
    nc.vector.bn_aggr(mv3[:], stat3[:])
    ss3 = _mkstats(nc, work, mv3, 512, "l3")
    psf3 = psS.tile([128, 2], F32, tag="small", name="psf3")
    nc.tensor.matmul(psf3[:1, :], ones125[:], ss3[:], start=True, stop=True)
    sbf3 = work.tile([1, 2], F32, tag="sbf3")
    nc.scalar.copy(sbf3[:], psf3[:1, :])
    nc.gpsimd.dma_start(d["st3_i"][:], sbf3[:])
    nc.gpsimd.collective_compute(
        "AllGather", ALU.bypass, replica_groups=RG,
        ins=[d["st3_i"][:]], outs=[d["st3_o"][:]])
    agg3 = work.tile([8, 2], F32, tag="agg3")
    nc.gpsimd.dma_start(agg3[:], d["st3_o"][:])
    psg3 = psS.tile([128, 2], F32, tag="small", name="psg3")
    nc.tensor.matmul(psg3[:1, :], ones8[:], agg3[:], start=True, stop=True)
    ssg3 = work.tile([1, 2], F32, tag="ssg3")
    nc.scalar.copy(ssg3[:], psg3[:1, :])
    mr3 = _mv_from_ss(nc, work, ssg3, B * P, "l3")
    scb1 = work.tile([1, 2], F32, tag="scb1")
    nc.vector.tensor_mul(scb1[:, 0:1], g3[:], mr3[:, 1:2])
    nc.vector.tensor_mul(scb1[:, 1:2], scb1[:, 0:1], mr3[:, 0:1])
    nc.vector.tensor_sub(scb1[:, 1:2], be3[:], scb1[:, 1:2])
    psb3 = psS.tile([128, 2], F32, tag="small", name="psb3")
    nc.tensor.matmul(psb3[:PH, :], ones1x[:, :PH], scb1[:], start=True,
                     stop=True)
    scb = work.tile([PH, 2], F32, tag="scb")
    nc.scalar.copy(scb[:], psb3[:PH, :])
    # relu(BN3) in place on PSUM, then exp -> attention numerators
    nc.scalar.activation(l3ps[:], l3ps[:], AF.Relu,
                         bias=scb[:, 1:2], scale=scb[:, 0:1])
    expP = bigY.tile([PH, 512], BF16, tag="expP")
    nc.scalar.activation(expP[:], l3ps[:], AF.Exp)
    expv = expP[:].rearrange("p (s h a) -> p h a s", s=64, h=2, a=4)
    # partial softmax denominators: sum over h (vector) then partitions (PE)
    zpart = work.tile([PH, 256], F32, tag="zpart")
    zpv = zpart[:].rearrange("p (a s) -> p a s", a=4, s=64)
    nc.vector.tensor_add(zpv, expv[:, 0], expv[:, 1])
    psz = psS.tile([128, 256], F32, tag="psz")
    nc.tensor.matmul(psz[:1, :], ones125[:], zpart[:], start=True, stop=True)
    zq = work.tile([1, 256], BF16, tag="zq")
    nc.scalar.copy(zq[:], psz[:1, :])
    # z into every shard's aux row of rs5_i (cols already in segment order)
    for cc in range(NCORES):
        nc.sync.dma_start(d["rs5_i"][cc * 129 + 128: cc * 129 + 129, :],
                          zq[:])

    psL3_cm.__exit__(None, None, None)
    psS_cm.__exit__(None, None, None)
    psA_cm.__exit__(None, None, None)
    bigX_cm.__exit__(None, None, None)

    # xa/h1/h2 are dead now -> their SBUF region hosts the fw1 tail chunk
    # and the small tail tiles.
    big2_cm = tc.tile_pool(name="big2", bufs=1)
    big2 = big2_cm.__enter__()
    fwBt = big2.tile([PH, C * 2 - FW_SPLIT, 1024], BF16, tag="fwB")
    nc.sync.dma_start(fwBt[:], fwv[:, FW_SPLIT:, :])

    # ---------------- FC1 (contraction-sharded, out [1024, 256] partial)
    psF_cm = tc.tile_pool(name="psF", bufs=1, space="PSUM")
    ptp_cm = tc.tile_pool(name="ptp", bufs=3)
    psF = psF_cm.__enter__()
    ptp = ptp_cm.__enter__()
    r1ps = [psF.tile([128, 256], F32, name=f"r1ps_{m}", tag=f"r1_{m}")
            for m in range(8)]
    NIT = C * 2
    for ch in range(C):
        for h in range(2):
            it = ch * 2 + h
            fw = (fwAt[:, it, :] if it < FW_SPLIT
                  else fwBt[:, it - FW_SPLIT, :])
            pt = ptp.tile([PH, 256], BF16, tag="pt", name=f"pt_{it}")
            ptv = pt[:].rearrange("p (a s) -> p a s", a=4, s=64)
            nc.vector.tensor_mul(ptv, xbv[:, ch, h], expv[:, h])
            for m in range(8):
                nc.tensor.matmul(
                    r1ps[m][:, :], fw[:, m * 128: (m + 1) * 128], pt[:],
                    start=(it == 0), stop=(it == NIT - 1))
    _rings = [nc.sync, nc.scalar]
    for m in range(8):
        r1sb = big2.tile([128, 256], BF16, tag="r1sb", name=f"r1sb_{m}",
                         bufs=2)
        nc.scalar.copy(r1sb[:], r1ps[m][:])
        _rings[m % 2].dma_start(d["rs5_i"][m * 129: m * 129 + 128, :],
                                r1sb[:])
    nc.gpsimd.collective_compute(
        "ReduceScatter", ALU.add, replica_groups=RG,
        ins=[d["rs5_i"][:]], outs=[d["rs5_o"][:]])

    ptp_cm.__exit__(None, None, None)
    psF_cm.__exit__(None, None, None)

    # ---------------- FC1 finish + FC2 + tail
    ps2_cm = tc.tile_pool(name="ps2", bufs=1, space="PSUM")
    ps2 = ps2_cm.__enter__()

    r1h = big2.tile([128, 256], BF16, tag="r1h")
    nc.sync.dma_start(r1h[:], d["rs5_o"][0:128, :])
    zrow = work.tile([1, 256], BF16, tag="zrow")
    nc.scalar.dma_start(zrow[:], d["rs5_o"][128:129, :])
    zrec = work.tile([1, 256], F32, tag="zrec")
    nc.vector.reciprocal(zrec[:], zrow[:])
    ps_z = ps2.tile([128, 256], F32, tag="zb")
    nc.tensor.matmul(ps_z[:], ones1x[:], zrec[:], start=True, stop=True)
    zinv = big2.tile([128, 256], BF16, tag="zinv")
    nc.scalar.copy(zinv[:], ps_z[:])
    r1 = big2.tile([128, 256], F32, tag="r1")
    nc.vector.tensor_mul(r1[:], r1h[:], zinv[:])
    # BN over segments (free dim), relu
    stf1 = work.tile([128, 6], F32, tag="stf1")
    nc.vector.bn_stats(stf1[:], r1[:])
    mvf1 = work.tile([128, 2], F32, tag="mvf1")
    nc.vector.bn_aggr(mvf1[:], stf1[:])
    epsf = work.tile([128, 1], F32, tag="epsf")
    nc.vector.memset(epsf[:], EPS_BN)
    nc.scalar.activation(mvf1[:, 1:2], mvf1[:, 1:2], AF.Sqrt, bias=epsf[:])
    nc.vector.reciprocal(mvf1[:, 1:2], mvf1[:, 1:2])
    fg1 = load("fg1s", [128, 1], pool=work)
    fbe1 = load("fbe1s", [128, 1], pool=work)
    scf1, bif1 = _scale_bias(nc, work, mvf1, fg1, fbe1, "f1")
    r1b = big2.tile([128, 256], BF16, tag="r1b")
    nc.scalar.activation(r1b[:], r1[:], AF.Relu, bias=bif1[:], scale=scf1[:])
    # FC2 partial
    fw2 = load("fw2t", [128, 256], BF16, pool=work)
    r2sb = big2.tile([128, 2, 256], F32, tag="r2sb")
    for m in range(2):
        ps_r2 = ps2.tile([128, 256], F32, tag=f"r2_{m}")
        nc.tensor.matmul(ps_r2[:], fw2[:, m * 128: (m + 1) * 128], r1b[:],
                         start=True, stop=True)
        nc.scalar.copy(r2sb[:, m, :], ps_r2[:])
        _rings[m].dma_start(d["ar6_i"][m * 128: (m + 1) * 128, :],
                            r2sb[:, m, :])
    nc.gpsimd.collective_compute(
        "AllReduce", ALU.add, replica_groups=RG,
        ins=[d["ar6_i"][:]], outs=[d["ar6_o"][:]])

    # tail: BN over segments per o2-row + relu (both blocks), L2-norm via
    # ones-matmul row-sums of squares (before the transposes), then transpose.
    fg2 = load("fg2t", [128, 2], pool=work)
    fbe2 = load("fbe2t", [128, 2], pool=work)
    r2s = []
    ps_n = ps2.tile([128, 256], F32, tag="nrm")
    for m in range(2):
        r2 = big2.tile([128, 256], F32, tag=f"r2_{m}", name=f"r2_{m}")
        _rings[m].dma_start(r2[:], d["ar6_o"][m * 128: (m + 1) * 128, :])
        stf2 = work.tile([128, 6], F32, tag=f"stf2_{m}")
        nc.vector.bn_stats(stf2[:], r2[:])
        mvf2 = work.tile([128, 2], F32, tag=f"mvf2_{m}")
        nc.vector.bn_aggr(mvf2[:], stf2[:])
        nc.scalar.activation(mvf2[:, 1:2], mvf2[:, 1:2], AF.Sqrt, bias=epsf[:])
        nc.vector.reciprocal(mvf2[:, 1:2], mvf2[:, 1:2])
        scf2, bif2 = _scale_bias(nc, work, mvf2,
                                 fg2[:, m: m + 1], fbe2[:, m: m + 1],
                                 f"f2_{m}")
        nc.scalar.activation(r2[:], r2[:], AF.Relu, bias=bif2[:], scale=scf2[:])
        sq = big2.tile([128, 256], F32, tag=f"sq_{m}", name=f"sq_{m}")
        nc.scalar.activation(sq[:], r2[:], AF.Square)
        nc.tensor.matmul(ps_n[:1, :], ones128[:], sq[:],
                         start=(m == 0), stop=(m == 1))
        r2s.append(r2)
    nrm = work.tile([1, 256], F32, tag="nrmrow")
    nc.scalar.activation(nrm[:], ps_n[:1, :], AF.Sqrt)
    nc.vector.tensor_scalar_max(nrm[:], nrm[:], 1e-12)
    nc.vector.reciprocal(nrm[:], nrm[:])
    ps_nb = ps2.tile([128, 256], F32, tag="nrmb")
    nc.tensor.matmul(ps_nb[:], ones1x[:], nrm[:], start=True, stop=True)
    nrmb = big2.tile([128, 256], F32, tag="nrmbs")
    nc.scalar.copy(nrmb[:], ps_nb[:])
    outT = big2.tile([128, 2, 256], F32, tag="outT")
    for m in range(2):
        nc.vector.tensor_mul(r2s[m][:], r2s[m][:], nrmb[:])
        for tt in range(2):
            ps_t = ps2.tile([128, 128], F32, tag="tailT",
                            name=f"tailT_{m}_{tt}", bufs=2)
            nc.tensor.transpose(ps_t[:], r2s[m][:, tt * 128: (tt + 1) * 128],
                                ident[:])
            nc.scalar.copy(outT[:, tt, m * 128: (m + 1) * 128], ps_t[:])
    for tt in range(2):
        _rings[tt].dma_start(d["out_final"][tt * 128: (tt + 1) * 128, :],
                             outT[:, tt, :])

    ps2_cm.__exit__(None, None, None)
    big2_cm.__exit__(None, None, None)
    fwA_cm.__exit__(None, None, None)
    work_cm.__exit__(None, None, None)
    bigY_cm.__exit__(None, None, None)
    sing_cm.__exit__(None, None, None)
    warm_cm.__exit__(None, None, None)


# ------------------------------------------------------------------ host side
def _prep_core(x3, fw1, c):
    xs = x3[:, PL * c: PL * (c + 1), :]                        # [256,250,32]
    arr = np.ascontiguousarray(xs.transpose(2, 0, 1))          # [32,256,250]
    xA4 = arr.reshape(C, 4, QF).transpose(1, 0, 2).reshape(128, QF)
    xb = xs.reshape(B, 2, PH, C).transpose(2, 3, 1, 0)         # [125,32,2,256]
    xB = np.ascontiguousarray(xb).reshape(PH, C * 2 * B)
    fw = fw1.reshape(1024, P, C)[:, PL * c: PL * (c + 1), :]
    fw = fw.reshape(1024, 2, PH, C).transpose(2, 3, 1, 0)      # [125,32,2,1024]
    fw1t = np.ascontiguousarray(fw).reshape(PH, C * 2 * 1024)
    bf = np.float16
    return (np.ascontiguousarray(xA4).astype(bf), xB.astype(bf),
            fw1t.astype(bf))


def _qrep(v, rows):
    out = np.zeros((128, 1), np.float32)
    for a in range(4):
        out[32 * a: 32 * a + rows, 0] = v
    return out


def _wdiag(w):
    """w [out,in] -> block-diagonal lhsT [128, 128]: block a (32x32) holds
    w.T in its top-left corner."""
    t = np.zeros((128, 128), np.float32)
    wt = w.T  # [in, out]
    for a in range(4):
        t[32 * a: 32 * a + wt.shape[0], 32 * a: 32 * a + wt.shape[1]] = wt
    return t


def kernel(**inputs):
    if "nc" not in _cache:
        _cache["nc"] = _build()
    nc = _cache["nc"]
    bf = np.float16

    g = {k: np.asarray(v, np.float32) for k, v in inputs.items()
         if k != "length"}
    x3 = g["x"].reshape(B, P, C)

    f1 = np.zeros((128, 16), np.float32)
    f2 = np.zeros((128, 8), np.float32)
    for a in range(4):
        f1[32 * a: 32 * a + 16, :] = np.eye(16, dtype=np.float32)
        f2[32 * a: 32 * a + 8, :] = np.eye(8, dtype=np.float32)
    f8_16 = np.zeros((128, 16), np.float32)
    f8_8 = np.zeros((64, 8), np.float32)
    for k in range(8):
        f8_16[16 * k: 16 * k + 16, :] = np.eye(16, dtype=np.float32)
        f8_8[8 * k: 8 * k + 8, :] = np.eye(8, dtype=np.float32)
    w3sp = np.zeros((128, 4), np.float32)
    for a in range(4):
        w3sp[32 * a: 32 * a + 8, a] = g["w3"][0, :]

    shared = {
        "w1D": _wdiag(g["w1"]).astype(bf),
        "w2D": _wdiag(g["w2"]).astype(bf),
        "w3sp": w3sp.astype(bf),
        "g1q": _qrep(g["g1"], 16), "be1q": _qrep(g["be1"], 16),
        "g2q": _qrep(g["g2"], 8), "be2q": _qrep(g["be2"], 8),
        "g3s": g["g3"].reshape(1, 1), "be3s": g["be3"].reshape(1, 1),
        "f1": f1, "ft1": np.ascontiguousarray(f1.T),
        "f2": f2, "ft2": np.ascontiguousarray(f2.T),
        "f8_16": f8_16, "f8_8": f8_8,
        "fg2t": np.ascontiguousarray(g["fg2"].reshape(2, 128).T),
        "fbe2t": np.ascontiguousarray(g["fbe2"].reshape(2, 128).T),
    }

    in_maps = []
    for c in range(NCORES):
        xA4, xB, fw1t = _prep_core(x3, g["fw1"], c)
        m = dict(shared)
        m["xA4"] = xA4
        m["xB"] = xB
        m["fw1t"] = fw1t
        m["fw2t"] = np.ascontiguousarray(
            g["fw2"][:, 128 * c: 128 * (c + 1)].T).astype(bf)
        m["fg1s"] = g["fg1"][128 * c: 128 * (c + 1)].reshape(128, 1)
        m["fbe1s"] = g["fbe1"][128 * c: 128 * (c + 1)].reshape(128, 1)
        in_maps.append(m)

    from concourse.bass_utils import run_bass_kernel_spmd

    res = run_bass_kernel_spmd(nc, in_maps, core_ids=list(range(NCORES)),
                               trace=bool(_cache.get("trace")))
    _cache["last_result"] = res
    return np.asarray(res.results[0]["out_final"], np.float32)


if __name__ == "__main__":
    nc = _build()
    print("build ok; instructions:",
          sum(len(bb.instructions) for bb in nc.main_func.blocks))


# revision 34
# speedup vs baseline: 1.0558x; 1.0558x over previous
"""Trainium2 Bass kernel for nn_FCGF_point_att3_sft_7000 (8 NeuronCores).

Model: pointwise attention MLP (32->16->8->1, BN+relu, BN stats over the full
512000-point batch), per-segment softmax over 2000 points, attention-weighted
pooling to [256, 64000], FC head 64000->1024->256 (BN+relu, stats over the
256-segment batch), final L2 row-normalize.

Sharding: points-within-segment. Core c owns points p in [250c, 250(c+1)) of
every segment. Stage A is data-parallel over points with AllGather'd BN stats;
fc1 is contraction-sharded (each core owns 8000 of the 64000 inputs and the
matching fw1 rows), summed via ReduceScatter whose per-shard aux row also
carries the softmax denominators; fc2 is contraction-sharded and finished with
an AllReduce; the tail is replicated.

Layout notes:
- Stage A layers 1-2 run in "quartered" A-orientation: x.T is [128, 16000]
  with the channels of free-quarter a on partitions [32a, 32a+32), matmuls
  use a block-diagonal lhsT so every chunk op runs 128 partitions wide.
- Layer 3 uses transpose-matmuls: lhsT = h2 point-chunk [128, 125], rhs =
  w3 spread over the 4 quarter blocks [128, 4] -> scores land point-major in
  a single PSUM bank [125, (seg64, half2, quarter4)].  relu/exp/softmax-z all
  run on that tiny tile; the old y3 eviction / repack / PE-transposes vanish.
- A zero-dependency warmup AllGather triggers at t~0 so the ~50us ncfw init
  overlaps the input DMAs and stage-A layer 1.
- fw1 streams as two big contiguous DMAs (96KB + 32KB per partition runs).
- ReduceScatter runs in fp16 (partials + softmax z), AllReduce in f32.

Training-mode BN is shift-invariant => conv/linear biases (b1,b2,b3,fb1,fb2)
drop out exactly; they are accepted and ignored.
"""

import sys

sys.path.insert(0, "/opt/trn_rl_repo")

import numpy as np

import concourse.bass as bass
import concourse.tile as tile
from concourse import mybir
from concourse.masks import make_identity

B = 256
P = 2000
C = 32
NCORES = 8
PL = P // NCORES           # 250
PH = PL // 2               # 125
NPTS = B * PL              # 64000 points per core
QF = NPTS // 4             # 16000 per quarter
NCH = 1000                 # stage-A eviction chunk (two 500-col matmuls)
NCHUNK = QF // NCH         # 16
FW_SPLIT = 44              # fc1 its in the first (big) weight DMA
EPS_BN = 1e-5
# Per-core BN statistics (skip the stats AllGathers).  Each core normalizes
# with moments of its own 64000 points; sampling error ~0.4% which largely
# cancels through the per-segment softmax.  Set False for exact global stats.
LOCAL_STATS = False
F32 = mybir.dt.float32
BF16 = mybir.dt.float16  # fp16: same speed as bf16, 8x lower rounding noise
RG = [list(range(NCORES))]
AF = mybir.ActivationFunctionType
ALU = mybir.AluOpType

_cache = {}


# ------------------------------------------------------------------ walrus fix
def _install_walrus_patch():
    """This container's walrus accepts only ONE semaphore wait per instruction.
    Spread Tile's end-of-kernel drain waits across single-wait nops, and split
    any instruction carrying >1 waits onto same-engine carrier nops."""
    if _cache.get("patched"):
        return
    from concourse.vector_clock import ScopedClock, VectorClock

    counter = [0]

    def split_waits(nc):
        for bb in nc.main_func.blocks:
            out = []
            changed = False
            for ins in bb.instructions:
                si = ins.sync_info
                waits = list(si.on_wait) if si and si.on_wait else []
                if len(waits) > 1:
                    changed = True
                    for w in waits[:-1]:
                        counter[0] += 1
                        out.append(mybir.InstNoOp(
                            name=f"I-wsplit-{counter[0]}",
                            engine=ins.engine, ins=[], outs=[],
                            sync_info=mybir.SyncInfo(on_wait=[w], on_update=[]),
                            bass_nofuse=True))
                    si.on_wait = waits[-1:]
                out.append(ins)
            if changed:
                try:
                    bb.instructions = out
                except Exception:
                    bb.instructions.clear()
                    for x in out:
                        bb.instructions.append(x)

    def _patched(self, tick_clock, wait_clock):
        nc = self.nc
        gc = tick_clock.global_clock
        n = len(gc)
        for i in range(n):
            if gc[i] > 0:
                vec = [0] * n
                vec[i] = gc[i]
                nop = nc.sync.nop(nofuse=True, hint=f"drain_wait_p{i}")
                wait_clock.add_sem_waits(
                    nop.ins, ScopedClock({None: VectorClock(vec)}))
        nc.sync.drain()
        nc.all_engine_barrier()
        assert self.sems is not None
        popped = nc._tile_sem_poison_stack.pop()
        assert popped is self._sem_poison
        nc.clear_and_free_semaphores(list(self.sems.allocated().values()))
        nc.all_engine_barrier()
        split_waits(nc)

    tile.TileContext._drain_and_barrier = _patched
    _cache["patched"] = True


# ------------------------------------------------------------------ bass build
def _build():
    _install_walrus_patch()
    nc = bass.Bass()

    def ein(name, shape, dt):
        return nc.dram_tensor(name, shape, dt, kind="ExternalInput")

    d = {}
    d["xA4"] = ein("xA4", [128, QF], BF16)
    d["xB"] = ein("xB", [PH, C * 2 * B], BF16)
    d["w1D"] = ein("w1D", [128, 128], BF16)
    d["w2D"] = ein("w2D", [128, 128], BF16)
    d["w3sp"] = ein("w3sp", [128, 4], BF16)
    for n in ("g1q", "be1q", "g2q", "be2q"):
        d[n] = ein(n, [128, 1], F32)
    d["g3s"] = ein("g3s", [1, 1], F32)
    d["be3s"] = ein("be3s", [1, 1], F32)
    d["f1"] = ein("f1", [128, 16], F32)
    d["ft1"] = ein("ft1", [16, 128], F32)
    d["f2"] = ein("f2", [128, 8], F32)
    d["ft2"] = ein("ft2", [8, 128], F32)
    d["f8_16"] = ein("f8_16", [128, 16], F32)
    d["f8_8"] = ein("f8_8", [64, 8], F32)
    d["fw1t"] = ein("fw1t", [PH, C * 2 * 1024], BF16)
    d["fw2t"] = ein("fw2t", [128, 256], BF16)
    d["fg1s"] = ein("fg1s", [128, 1], F32)
    d["fbe1s"] = ein("fbe1s", [128, 1], F32)
    d["fg2t"] = ein("fg2t", [128, 2], F32)
    d["fbe2t"] = ein("fbe2t", [128, 2], F32)
    d["out_final"] = nc.dram_tensor("out_final", [256, 256], F32,
                                    kind="ExternalOutput")
    # collective bounce buffers
    d["warm_i"] = nc.dram_tensor("warm_i", [1, 4], F32)
    d["warm_o"] = nc.dram_tensor("warm_o", [8, 4], F32)
    d["st1_i"] = nc.dram_tensor("st1_i", [16, 2], F32)
    d["st1_o"] = nc.dram_tensor("st1_o", [128, 2], F32)
    d["st2_i"] = nc.dram_tensor("st2_i", [8, 2], F32)
    d["st2_o"] = nc.dram_tensor("st2_o", [64, 2], F32)
    d["st3_i"] = nc.dram_tensor("st3_i", [1, 2], F32)
    d["st3_o"] = nc.dram_tensor("st3_o", [8, 2], F32)
    d["rs5_i"] = nc.dram_tensor("rs5_i", [NCORES * 129, 256], BF16)
    d["rs5_o"] = nc.dram_tensor("rs5_o", [129, 256], BF16)
    d["ar6_i"] = nc.dram_tensor("ar6_i", [256, 256], BF16)
    d["ar6_o"] = nc.dram_tensor("ar6_o", [256, 256], BF16)

    with tile.TileContext(nc) as tc:
        _body(nc, tc, d)
    return nc


def _mkstats(nc, pool, mv, count, name):
    """mv [p,2]=(mean,var) -> (sum,sumsq) [p,2]."""
    p = mv.shape[0]
    ss = pool.tile([p, 2], F32, tag=f"ss_{name}")
    nc.vector.tensor_mul(ss[:, 1:2], mv[:, 0:1], mv[:, 0:1])
    nc.vector.tensor_add(ss[:, 1:2], ss[:, 1:2], mv[:, 1:2])
    nc.scalar.mul(ss[:, 0:1], mv[:, 0:1], float(count))
    nc.scalar.mul(ss[:, 1:2], ss[:, 1:2], float(count))
    return ss


def _mv_from_ss(nc, pool, ss, count, name):
    """(sum,sumsq) [p,2] over count -> (mean, rstd) [p,2]."""
    p = ss.shape[0]
    mr = pool.tile([p, 2], F32, tag=f"mr_{name}")
    epst = pool.tile([p, 1], F32, tag=f"eps_{name}")
    nc.vector.memset(epst[:], EPS_BN)
    nc.scalar.mul(mr[:, 0:1], ss[:, 0:1], 1.0 / count)
    nc.scalar.mul(mr[:, 1:2], ss[:, 1:2], 1.0 / count)
    m2 = pool.tile([p, 1], F32, tag=f"m2_{name}")
    nc.vector.tensor_mul(m2[:], mr[:, 0:1], mr[:, 0:1])
    nc.vector.tensor_sub(mr[:, 1:2], mr[:, 1:2], m2[:])
    nc.scalar.activation(mr[:, 1:2], mr[:, 1:2], AF.Sqrt, bias=epst[:])
    nc.vector.reciprocal(mr[:, 1:2], mr[:, 1:2])
    return mr


def _scale_bias(nc, pool, mrq, g, be, name):
    """scale = g*rstd ; bias = be - scale*mean  (all [p,1] per-partition)."""
    p = mrq.shape[0]
    sc = pool.tile([p, 1], F32, tag=f"sc_{name}")
    bi = pool.tile([p, 1], F32, tag=f"bi_{name}")
    nc.vector.tensor_mul(sc[:], g[:], mrq[:, 1:2])
    nc.vector.tensor_mul(bi[:], sc[:], mrq[:, 0:1])
    nc.vector.tensor_sub(bi[:], be[:], bi[:])
    return sc, bi


def _body(nc, tc, d):
    sing_cm = tc.tile_pool(name="sing", bufs=1)
    bigY_cm = tc.tile_pool(name="bigY", bufs=1)   # xb + exp tiles: live to FC1
    work_cm = tc.tile_pool(name="work", bufs=1)
    fwA_cm = tc.tile_pool(name="fwA", bufs=1)
    bigX_cm = tc.tile_pool(name="bigX", bufs=1)   # xa/h2 + h1: dies after L3
    psA_cm = tc.tile_pool(name="psA", bufs=2, space="PSUM")
    psS_cm = tc.tile_pool(name="psS", bufs=1, space="PSUM")
    sing = sing_cm.__enter__()
    bigY = bigY_cm.__enter__()
    work = work_cm.__enter__()
    fwA_p = fwA_cm.__enter__()
    bigX = bigX_cm.__enter__()
    psA = psA_cm.__enter__(); psS = psS_cm.__enter__()

    # ---------------- big loads FIRST: the HWDGE issue path drains roughly
    # in issue order, so xa (gates L1) and xb go before everything else.
    xa = bigX.tile([128, QF], BF16, tag="slotA")      # xa -> (dead) -> h2
    nc.sync.dma_start(xa[:], d["xA4"][:])
    xb = bigY.tile([PH, C * 2 * B], BF16, tag="xb")
    nc.scalar.dma_start(xb[:], d["xB"][:])
    xbv = xb[:].rearrange("p (c h a s) -> p c h a s", c=C, h=2, a=4, s=64)

    # ---------------- constants (sync ring; small)
    def load(name, shape, dt=F32, pool=sing):
        t = pool.tile(shape, dt, tag=name)
        nc.sync.dma_start(t[:], d[name][:])
        return t

    w1D = load("w1D", [128, 128], BF16)
    w2D = load("w2D", [128, 128], BF16)
    w3sp = load("w3sp", [128, 4], BF16)
    f1s = load("f1", [128, 16])
    ft1s = load("ft1", [16, 128])
    f2s = load("f2", [128, 8])
    ft2s = load("ft2", [8, 128])
    f8_16s = load("f8_16", [128, 16])
    f8_8s = load("f8_8", [64, 8])
    g1 = load("g1q", [128, 1]); be1 = load("be1q", [128, 1])
    g2 = load("g2q", [128, 1]); be2 = load("be2q", [128, 1])
    g3 = load("g3s", [1, 1]); be3 = load("be3s", [1, 1])
    ones128 = sing.tile([128, 1], F32)
    nc.vector.memset(ones128[:], 1.0)
    ones8 = sing.tile([8, 1], F32)
    nc.vector.memset(ones8[:], 1.0)
    ones125 = sing.tile([PH, 1], F32)
    nc.vector.memset(ones125[:], 1.0)
    ones1x = sing.tile([1, 128], F32)
    nc.vector.memset(ones1x[:], 1.0)
    ident = sing.tile([128, 128], F32)
    make_identity(nc, ident[:])
    ident16 = sing.tile([128, 128], BF16)
    make_identity(nc, ident16[:])

    # fw1 head: chunked 8-its per InstDMACopy on alternating rings, queued
    # behind xa/xb/consts.
    fwv = d["fw1t"][:].rearrange("p (i o) -> p i o", i=C * 2, o=1024)
    fwAt = fwA_p.tile([PH, FW_SPLIT, 1024], BF16, tag="fwA")
    _fwr = [nc.sync, nc.scalar]
    for k in range(0, FW_SPLIT, 8):
        hi = min(k + 8, FW_SPLIT)
        _fwr[(k // 8) % 2].dma_start(fwAt[:, k:hi, :], fwv[:, k:hi, :])


    def stage_layer(rhs_src, wD, fold, foldT, f8fold, st_i, st_o,
                    gq, beq, name, out_tag):
        """Quartered A-orientation layer: matmuls -> raw evict (scalar) +
        bn_stats (vector, from PSUM), fold + AllGather stats, then BN+relu
        applied in place, split scalar/vector."""
        y = bigX.tile([128, QF], BF16, tag=out_tag, name=f"y_{name}")
        stat = work.tile([128, 2 * NCHUNK, 6], F32, tag=f"stat_{name}")
        for j in range(NCHUNK):
            ps = psA.tile([128, 1024], F32, tag="psA", name=f"ps_{name}_{j}")
            base = j * NCH
            nc.tensor.matmul(ps[:, 0:500], wD[:], rhs_src[:, base:base + 500],
                             start=True, stop=True)
            nc.tensor.matmul(ps[:, 512:1012], wD[:],
                             rhs_src[:, base + 500:base + 1000],
                             start=True, stop=True)
            pv = ps[:].rearrange("p (k c) -> p k c", k=2, c=512)[:, :, 0:500]
            nc.scalar.copy(
                y[:, base:base + NCH].rearrange("p (k c) -> p k c", k=2,
                                                c=500), pv)
            nc.vector.bn_stats(stat[:, 2 * j, :], ps[:, 0:500])
            nc.vector.bn_stats(stat[:, 2 * j + 1, :], ps[:, 512:1012])
        mv = work.tile([128, 2], F32, tag=f"mv_{name}")
        nc.vector.bn_aggr(mv[:], stat[:])
        ss = _mkstats(nc, work, mv, QF, name)
        nfold = fold.shape[1]
        psf = psS.tile([128, 2], F32, tag="small", name=f"psf_{name}")
        nc.tensor.matmul(psf[:nfold, :], fold[:], ss[:], start=True, stop=True)
        sbf = work.tile([nfold, 2], F32, tag=f"sbf_{name}")
        nc.scalar.copy(sbf[:], psf[:nfold, :])
        if LOCAL_STATS:
            mr = _mv_from_ss(nc, work, sbf, B * P // NCORES, name)
        else:
            nc.gpsimd.dma_start(st_i[:], sbf[:])
            nc.gpsimd.collective_compute(
                "AllGather", ALU.bypass, replica_groups=RG,
                ins=[st_i[:]], outs=[st_o[:]])
            agg = work.tile([nfold * NCORES, 2], F32, tag=f"agg_{name}")
            nc.gpsimd.dma_start(agg[:], st_o[:])
            psg = psS.tile([128, 2], F32, tag="small", name=f"psg_{name}")
            nc.tensor.matmul(psg[:nfold, :], f8fold[:], agg[:], start=True,
                             stop=True)
            ssg = work.tile([nfold, 2], F32, tag=f"ssg_{name}")
            nc.scalar.copy(ssg[:], psg[:nfold, :])
            mr = _mv_from_ss(nc, work, ssg, B * P, name)
        psb = psS.tile([128, 2], F32, tag="small", name=f"psb_{name}")
        nc.tensor.matmul(psb[:], foldT[:], mr[:], start=True, stop=True)
        mrq = work.tile([128, 2], F32, tag=f"mrq_{name}")
        nc.scalar.copy(mrq[:], psb[:])
        sc, bi = _scale_bias(nc, work, mrq, gq, beq, name)
        # relu in place: scalar takes the first chunks, vector the rest
        NSC = 6
        for j in range(NSC):
            sl = slice(j * NCH, (j + 1) * NCH)
            nc.scalar.activation(y[:, sl], y[:, sl], AF.Relu,
                                 bias=bi[:], scale=sc[:])
        for j in range(NSC, NCHUNK):
            sl = slice(j * NCH, (j + 1) * NCH)
            nc.vector.tensor_scalar(y[:, sl], y[:, sl], sc[:], bi[:],
                                    ALU.mult, ALU.add)
            nc.vector.tensor_scalar_max(y[:, sl], y[:, sl], 0.0)
        return y

    # ---------------- stage A layers 1 & 2
    h1 = stage_layer(xa, w1D, f1s, ft1s, f8_16s,
                     d["st1_i"], d["st1_o"], g1, be1, "l1", "slotB")
    # h2 reuses slot A (xa dead after L1 matmuls)
    h2 = stage_layer(h1, w2D, f2s, ft2s, f8_8s,
                     d["st2_i"], d["st2_o"], g2, be2, "l2", "slotA")

    # ---------------- layer 3 via transpose-matmuls: scores point-major.
    # lhsT = h2[:, 125c:125c+125] (K=128 channel-partitions, M=125 points),
    # rhs = w3 spread [128, 4] (col a = w3 in quarter-a rows) ->
    # psL3[:, 4c+a] = score of quarter a's point 125c+p.
    # Free-dim layout: c = (s, h) with s in 0..63, h in 0..1; col = 8s+4h+a.
    psL3_cm = tc.tile_pool(name="psL3", bufs=1, space="PSUM")
    psL3 = psL3_cm.__enter__()
    l3ps = psL3.tile([PH, 512], F32, tag="l3ps")
    for cgrp in range(128):
        nc.tensor.matmul(l3ps[:, 4 * cgrp: 4 * cgrp + 4],
                         h2[:, 125 * cgrp: 125 * cgrp + 125],
                         w3sp[:], start=True, stop=True)
    # BN3 stats over all points (125*512 = 64000 local)
    stat3 = work.tile([PH, 6], F32, tag="stat3")
    nc.vector.bn_stats(stat3[:], l3ps[:])
    mv3 = work.tile([PH, 2], F32, tag="mv3")
    nc.vector.bn_aggr(mv3[:], stat3[:])
    ss3 = _mkstats(nc, work, mv3, 512, "l3")
    psf3 = psS.tile([128, 2], F32, tag="small", name="psf3")
    nc.tensor.matmul(psf3[:1, :], ones125[:], ss3[:], start=True, stop=True)
    sbf3 = work.tile([1, 2], F32, tag="sbf3")
    nc.scalar.copy(sbf3[:], psf3[:1, :])
    if LOCAL_STATS:
        mr3 = _mv_from_ss(nc, work, sbf3, B * P // NCORES, "l3")
    else:
        nc.gpsimd.dma_start(d["st3_i"][:], sbf3[:])
        nc.gpsimd.collective_compute(
            "AllGather", ALU.bypass, replica_groups=RG,
            ins=[d["st3_i"][:]], outs=[d["st3_o"][:]])
        agg3 = work.tile([8, 2], F32, tag="agg3")
        nc.gpsimd.dma_start(agg3[:], d["st3_o"][:])
        psg3 = psS.tile([128, 2], F32, tag="small", name="psg3")
        nc.tensor.matmul(psg3[:1, :], ones8[:], agg3[:], start=True, stop=True)
        ssg3 = work.tile([1, 2], F32, tag="ssg3")
        nc.scalar.copy(ssg3[:], psg3[:1, :])
        mr3 = _mv_from_ss(nc, work, ssg3, B * P, "l3")
    scb1 = work.tile([1, 2], F32, tag="scb1")
    nc.vector.tensor_mul(scb1[:, 0:1], g3[:], mr3[:, 1:2])
    nc.vector.tensor_mul(scb1[:, 1:2], scb1[:, 0:1], mr3[:, 0:1])
    nc.vector.tensor_sub(scb1[:, 1:2], be3[:], scb1[:, 1:2])
    psb3 = psS.tile([128, 2], F32, tag="small", name="psb3")
    nc.tensor.matmul(psb3[:PH, :], ones1x[:, :PH], scb1[:], start=True,
                     stop=True)
    scb = work.tile([PH, 2], F32, tag="scb")
    nc.scalar.copy(scb[:], psb3[:PH, :])
    # relu(BN3) in place on PSUM, then exp -> attention numerators
    nc.scalar.activation(l3ps[:], l3ps[:], AF.Relu,
                         bias=scb[:, 1:2], scale=scb[:, 0:1])
    expP = bigY.tile([PH, 512], BF16, tag="expP")
    nc.scalar.activation(expP[:], l3ps[:], AF.Exp)
    expv = expP[:].rearrange("p (s h a) -> p h a s", s=64, h=2, a=4)
    # partial softmax denominators: sum over h (vector) then partitions (PE)
    zpart = work.tile([PH, 256], F32, tag="zpart")
    zpv = zpart[:].rearrange("p (a s) -> p a s", a=4, s=64)
    nc.vector.tensor_add(zpv, expv[:, 0], expv[:, 1])
    psz = psS.tile([128, 256], F32, tag="psz")
    nc.tensor.matmul(psz[:1, :], ones125[:], zpart[:], start=True, stop=True)
    zq = work.tile([1, 256], BF16, tag="zq")
    nc.scalar.copy(zq[:], psz[:1, :])
    # z into every shard's aux row of rs5_i (cols already in segment order)
    for cc in range(NCORES):
        nc.sync.dma_start(d["rs5_i"][cc * 129 + 128: cc * 129 + 129, :],
                          zq[:])

    psL3_cm.__exit__(None, None, None)
    psS_cm.__exit__(None, None, None)
    psA_cm.__exit__(None, None, None)
    bigX_cm.__exit__(None, None, None)

    # xa/h1/h2 are dead now -> their SBUF region hosts the fw1 tail chunk
    # and the small tail tiles.
    big2_cm = tc.tile_pool(name="big2", bufs=1)
    big2 = big2_cm.__enter__()
    fwBt = big2.tile([PH, C * 2 - FW_SPLIT, 1024], BF16, tag="fwB")
    for k in range(0, C * 2 - FW_SPLIT, 8):
        hi = min(k + 8, C * 2 - FW_SPLIT)
        _fwr2 = [nc.sync, nc.scalar][(k // 8) % 2]
        _fwr2.dma_start(fwBt[:, k:hi, :], fwv[:, FW_SPLIT + k:FW_SPLIT + hi, :])

    # ---------------- FC1 (contraction-sharded, out [1024, 256] partial)
    psF_cm = tc.tile_pool(name="psF", bufs=1, space="PSUM")
    ptp_cm = tc.tile_pool(name="ptp", bufs=3)
    psF = psF_cm.__enter__()
    ptp = ptp_cm.__enter__()
    # pt-stationary "swap" arrangement: lhsT = a 128-segment half of pt,
    # rhs = the full 1024-wide fw row (fp16 moving operand) -> out is
    # [seg-half, fc1out] in PSUM; 2 matmuls per it instead of 8 (each
    # InstMatmult carries a fused LDWEIGHTS here, so fewer/wider wins).
    r1ps = [psF.tile([128, 1024], F32, name=f"r1ps_{hh}", tag=f"r1_{hh}")
            for hh in range(2)]
    NIT = C * 2
    for ch in range(C):
        for h in range(2):
            it = ch * 2 + h
            fw = (fwAt[:, it, :] if it < FW_SPLIT
                  else fwBt[:, it - FW_SPLIT, :])
            pt = ptp.tile([PH, 256], BF16, tag="pt", name=f"pt_{it}")
            ptv = pt[:].rearrange("p (a s) -> p a s", a=4, s=64)
            nc.vector.tensor_mul(ptv, xbv[:, ch, h], expv[:, h])
            for hh in range(2):
                for q in range(2):
                    nc.tensor.matmul(
                        r1ps[hh][:, q * 512: q * 512 + 512],
                        pt[:, hh * 128: hh * 128 + 128],
                        fw[:, q * 512: q * 512 + 512],
                        start=(it == 0), stop=(it == NIT - 1))
    # transpose [seg, out] -> [out, seg] before staging (RS shards are
    # out-major).  Evict each half to fp16, then 16 PE transposes.
    # per-block evict -> PE transpose -> stage, pipelined across scalar /
    # vector / tensor; everything fits in psF (r1ps 4 banks + fc1T 2).
    _rings = [nc.sync, nc.scalar]
    r1fl = [None, None]
    for hh in range(2):
        r1fl[hh] = big2.tile([128, 1024], BF16, tag=f"r1fl_{hh}",
                             name=f"r1fl_{hh}")
    for m in range(8):
        for hh in range(2):
            blk = slice(m * 128, m * 128 + 128)
            if (m + hh) % 2 == 0:
                nc.scalar.copy(r1fl[hh][:, blk], r1ps[hh][:, blk])
            else:
                nc.vector.tensor_copy(r1fl[hh][:, blk], r1ps[hh][:, blk])
    for m in range(8):
        r1sb = big2.tile([128, 256], BF16, tag="r1sb", name=f"r1sb_{m}",
                         bufs=2)
        for hh in range(2):
            ps_t = psF.tile([128, 128], BF16, tag="fc1T",
                            name=f"fc1T_{m}_{hh}", bufs=2)
            nc.tensor.transpose(ps_t[:], r1fl[hh][:, m * 128: m * 128 + 128],
                                ident16[:])
            nc.scalar.copy(r1sb[:, hh * 128: hh * 128 + 128], ps_t[:])
        _rings[m % 2].dma_start(d["rs5_i"][m * 129: m * 129 + 128, :],
                                r1sb[:])
    nc.gpsimd.collective_compute(
        "ReduceScatter", ALU.add, replica_groups=RG,
        ins=[d["rs5_i"][:]], outs=[d["rs5_o"][:]])

    ptp_cm.__exit__(None, None, None)
    psF_cm.__exit__(None, None, None)

    # ---------------- FC1 finish + FC2 + tail
    ps2_cm = tc.tile_pool(name="ps2", bufs=1, space="PSUM")
    ps2 = ps2_cm.__enter__()

    r1h = big2.tile([128, 256], BF16, tag="r1h")
    nc.sync.dma_start(r1h[:], d["rs5_o"][0:128, :])
    zrow = work.tile([1, 256], BF16, tag="zrow")
    nc.scalar.dma_start(zrow[:], d["rs5_o"][128:129, :])
    zrec = work.tile([1, 256], F32, tag="zrec")
    nc.vector.reciprocal(zrec[:], zrow[:])
    ps_z = ps2.tile([128, 256], F32, tag="nrmb", name="zb")
    nc.tensor.matmul(ps_z[:], ones1x[:], zrec[:], start=True, stop=True)
    zinv = big2.tile([128, 256], BF16, tag="zinv")
    nc.scalar.copy(zinv[:], ps_z[:])
    r1 = big2.tile([128, 256], F32, tag="r1")
    nc.vector.tensor_mul(r1[:], r1h[:], zinv[:])
    # BN over segments (free dim), relu
    stf1 = work.tile([128, 6], F32, tag="stf1")
    nc.vector.bn_stats(stf1[:], r1[:])
    mvf1 = work.tile([128, 2], F32, tag="mvf1")
    nc.vector.bn_aggr(mvf1[:], stf1[:])
    epsf = work.tile([128, 1], F32, tag="epsf")
    nc.vector.memset(epsf[:], EPS_BN)
    nc.scalar.activation(mvf1[:, 1:2], mvf1[:, 1:2], AF.Sqrt, bias=epsf[:])
    nc.vector.reciprocal(mvf1[:, 1:2], mvf1[:, 1:2])
    fg1 = load("fg1s", [128, 1], pool=work)
    fbe1 = load("fbe1s", [128, 1], pool=work)
    scf1, bif1 = _scale_bias(nc, work, mvf1, fg1, fbe1, "f1")
    r1b = big2.tile([128, 256], BF16, tag="r1b")
    nc.scalar.activation(r1b[:], r1[:], AF.Relu, bias=bif1[:], scale=scf1[:])
    # FC2 partial
    fw2 = load("fw2t", [128, 256], BF16, pool=work)
    r2sb = big2.tile([128, 2, 256], BF16, tag="r2sb")
    for m in range(2):
        ps_r2 = ps2.tile([128, 256], F32, tag=f"r2_{m}")
        nc.tensor.matmul(ps_r2[:], fw2[:, m * 128: (m + 1) * 128], r1b[:],
                         start=True, stop=True)
        nc.scalar.copy(r2sb[:, m, :], ps_r2[:])
        _rings[m].dma_start(d["ar6_i"][m * 128: (m + 1) * 128, :],
                            r2sb[:, m, :])
    nc.gpsimd.collective_compute(
        "AllReduce", ALU.add, replica_groups=RG,
        ins=[d["ar6_i"][:]], outs=[d["ar6_o"][:]])

    # tail: BN over segments per o2-row + relu (both blocks), L2-norm via
    # ones-matmul row-sums of squares (before the transposes), then transpose.
    fg2 = load("fg2t", [128, 2], pool=work)
    fbe2 = load("fbe2t", [128, 2], pool=work)
    r2s = []
    ps_n = ps2.tile([128, 256], F32, tag="nrm")
    for m in range(2):
        r2 = big2.tile([128, 256], BF16, tag=f"r2_{m}", name=f"r2_{m}")
        _rings[m].dma_start(r2[:], d["ar6_o"][m * 128: (m + 1) * 128, :])
        stf2 = work.tile([128, 6], F32, tag=f"stf2_{m}")
        nc.vector.bn_stats(stf2[:], r2[:])
        mvf2 = work.tile([128, 2], F32, tag=f"mvf2_{m}")
        nc.vector.bn_aggr(mvf2[:], stf2[:])
        nc.scalar.activation(mvf2[:, 1:2], mvf2[:, 1:2], AF.Sqrt, bias=epsf[:])
        nc.vector.reciprocal(mvf2[:, 1:2], mvf2[:, 1:2])
        scf2, bif2 = _scale_bias(nc, work, mvf2,
                                 fg2[:, m: m + 1], fbe2[:, m: m + 1],
                                 f"f2_{m}")
        nc.scalar.activation(r2[:], r2[:], AF.Relu, bias=bif2[:], scale=scf2[:])
        sq = big2.tile([128, 256], F32, tag=f"sq_{m}", name=f"sq_{m}")
        nc.scalar.activation(sq[:], r2[:], AF.Square)
        nc.tensor.matmul(ps_n[:1, :], ones128[:], sq[:],
                         start=(m == 0), stop=(m == 1))
        r2s.append(r2)
    nrm = work.tile([1, 256], F32, tag="nrmrow")
    nc.scalar.activation(nrm[:], ps_n[:1, :], AF.Sqrt)
    nc.vector.tensor_scalar_max(nrm[:], nrm[:], 1e-12)
    nc.vector.reciprocal(nrm[:], nrm[:])
    ps_nb = ps2.tile([128, 256], F32, tag="nrmb")
    nc.tensor.matmul(ps_nb[:], ones1x[:], nrm[:], start=True, stop=True)
    nrmb = big2.tile([128, 256], BF16, tag="nrmbs")
    nc.scalar.copy(nrmb[:], ps_nb[:])
    outT = big2.tile([128, 2, 256], F32, tag="outT")
    for m in range(2):
        nc.vector.tensor_mul(r2s[m][:], r2s[m][:], nrmb[:])
        for tt in range(2):
            ps_t = ps2.tile([128, 128], BF16, tag="tailT",
                            name=f"tailT_{m}_{tt}", bufs=2)
            nc.tensor.transpose(ps_t[:], r2s[m][:, tt * 128: (tt + 1) * 128],
                                ident16[:])
            nc.scalar.copy(outT[:, tt, m * 128: (m + 1) * 128], ps_t[:])
    for tt in range(2):
        _rings[tt].dma_start(d["out_final"][tt * 128: (tt + 1) * 128, :],
                             outT[:, tt, :])

    ps2_cm.__exit__(None, None, None)
    big2_cm.__exit__(None, None, None)
    fwA_cm.__exit__(None, None, None)
    work_cm.__exit__(None, None, None)
    bigY_cm.__exit__(None, None, None)
    sing_cm.__exit__(None, None, None)


# ------------------------------------------------------------------ host side
def _prep_core(x3, fw1, c):
    xs = x3[:, PL * c: PL * (c + 1), :]                        # [256,250,32]
    arr = np.ascontiguousarray(xs.transpose(2, 0, 1))          # [32,256,250]
    xA4 = arr.reshape(C, 4, QF).transpose(1, 0, 2).reshape(128, QF)
    xb = xs.reshape(B, 2, PH, C).transpose(2, 3, 1, 0)         # [125,32,2,256]
    xB = np.ascontiguousarray(xb).reshape(PH, C * 2 * B)
    fw = fw1.reshape(1024, P, C)[:, PL * c: PL * (c + 1), :]
    fw = fw.reshape(1024, 2, PH, C).transpose(2, 3, 1, 0)      # [125,32,2,1024]
    fw1t = np.ascontiguousarray(fw).reshape(PH, C * 2 * 1024)
    bf = np.float16
    return (np.ascontiguousarray(xA4).astype(bf), xB.astype(bf),
            fw1t.astype(bf))


def _qrep(v, rows):
    out = np.zeros((128, 1), np.float32)
    for a in range(4):
        out[32 * a: 32 * a + rows, 0] = v
    return out


def _wdiag(w):
    """w [out,in] -> block-diagonal lhsT [128, 128]: block a (32x32) holds
    w.T in its top-left corner."""
    t = np.zeros((128, 128), np.float32)
    wt = w.T  # [in, out]
    for a in range(4):
        t[32 * a: 32 * a + wt.shape[0], 32 * a: 32 * a + wt.shape[1]] = wt
    return t


def kernel(**inputs):
    if "nc" not in _cache:
        _cache["nc"] = _build()
    nc = _cache["nc"]
    bf = np.float16

    g = {k: np.asarray(v, np.float32) for k, v in inputs.items()
         if k != "length"}
    x3 = g["x"].reshape(B, P, C)

    f1 = np.zeros((128, 16), np.float32)
    f2 = np.zeros((128, 8), np.float32)
    for a in range(4):
        f1[32 * a: 32 * a + 16, :] = np.eye(16, dtype=np.float32)
        f2[32 * a: 32 * a + 8, :] = np.eye(8, dtype=np.float32)
    f8_16 = np.zeros((128, 16), np.float32)
    f8_8 = np.zeros((64, 8), np.float32)
    for k in range(8):
        f8_16[16 * k: 16 * k + 16, :] = np.eye(16, dtype=np.float32)
        f8_8[8 * k: 8 * k + 8, :] = np.eye(8, dtype=np.float32)
    w3sp = np.zeros((128, 4), np.float32)
    for a in range(4):
        w3sp[32 * a: 32 * a + 8, a] = g["w3"][0, :]

    shared = {
        "w1D": _wdiag(g["w1"]).astype(bf),
        "w2D": _wdiag(g["w2"]).astype(bf),
        "w3sp": w3sp.astype(bf),
        "g1q": _qrep(g["g1"], 16), "be1q": _qrep(g["be1"], 16),
        "g2q": _qrep(g["g2"], 8), "be2q": _qrep(g["be2"], 8),
        "g3s": g["g3"].reshape(1, 1), "be3s": g["be3"].reshape(1, 1),
        "f1": f1, "ft1": np.ascontiguousarray(f1.T),
        "f2": f2, "ft2": np.ascontiguousarray(f2.T),
        "f8_16": f8_16, "f8_8": f8_8,
        "fg2t": np.ascontiguousarray(g["fg2"].reshape(2, 128).T),
        "fbe2t": np.ascontiguousarray(g["fbe2"].reshape(2, 128).T),
    }

    in_maps = []
    for c in range(NCORES):
        xA4, xB, fw1t = _prep_core(x3, g["fw1"], c)
        m = dict(shared)
        m["xA4"] = xA4
        m["xB"] = xB
        m["fw1t"] = fw1t
        m["fw2t"] = np.ascontiguousarray(
            g["fw2"][:, 128 * c: 128 * (c + 1)].T).astype(bf)
        m["fg1s"] = g["fg1"][128 * c: 128 * (c + 1)].reshape(128, 1)
        m["fbe1s"] = g["fbe1"][128 * c: 128 * (c + 1)].reshape(128, 1)
        in_maps.append(m)

    from concourse.bass_utils import run_bass_kernel_spmd

    res = run_bass_kernel_spmd(nc, in_maps, core_ids=list(range(NCORES)),
                               trace=bool(_cache.get("trace")))
    _cache["last_result"] = res
    return np.asarray(res.results[0]["out_final"], np.float32)


if __name__ == "__main__":
    nc = _build()
    print("build ok; instructions:",
          sum(len(bb.instructions) for bb in nc.main_func.blocks))


# revision 35
# speedup vs baseline: 1.1363x; 1.0762x over previous
"""Trainium2 Bass kernel for nn_FCGF_point_att3_sft_7000 (8 NeuronCores).

Model: pointwise attention MLP (32->16->8->1, BN+relu, BN stats over the full
512000-point batch), per-segment softmax over 2000 points, attention-weighted
pooling to [256, 64000], FC head 64000->1024->256 (BN+relu, stats over the
256-segment batch), final L2 row-normalize.

Sharding: points-within-segment. Core c owns points p in [250c, 250(c+1)) of
every segment. Stage A is data-parallel over points with AllGather'd BN stats;
fc1 is contraction-sharded (each core owns 8000 of the 64000 inputs and the
matching fw1 rows), summed via ReduceScatter whose per-shard aux row also
carries the softmax denominators; fc2 is contraction-sharded and finished with
an AllReduce; the tail is replicated.

Layout notes:
- Stage A layers 1-2 run in "quartered" A-orientation: x.T is [128, 16000]
  with the channels of free-quarter a on partitions [32a, 32a+32), matmuls
  use a block-diagonal lhsT so every chunk op runs 128 partitions wide.
- Layer 3 uses transpose-matmuls: lhsT = h2 point-chunk [128, 125], rhs =
  w3 spread over the 4 quarter blocks [128, 4] -> scores land point-major in
  a single PSUM bank [125, (seg64, half2, quarter4)].  relu/exp/softmax-z all
  run on that tiny tile; the old y3 eviction / repack / PE-transposes vanish.
- A zero-dependency warmup AllGather triggers at t~0 so the ~50us ncfw init
  overlaps the input DMAs and stage-A layer 1.
- fw1 streams as two big contiguous DMAs (96KB + 32KB per partition runs).
- ReduceScatter runs in fp16 (partials + softmax z), AllReduce in f32.

Training-mode BN is shift-invariant => conv/linear biases (b1,b2,b3,fb1,fb2)
drop out exactly; they are accepted and ignored.
"""

import sys

sys.path.insert(0, "/opt/trn_rl_repo")

import numpy as np

import concourse.bass as bass
import concourse.tile as tile
from concourse import mybir
from concourse.masks import make_identity

B = 256
P = 2000
C = 32
NCORES = 8
PL = P // NCORES           # 250
PH = PL // 2               # 125
NPTS = B * PL              # 64000 points per core
QF = NPTS // 4             # 16000 per quarter
NCH = 1000                 # stage-A eviction chunk (two 500-col matmuls)
NCHUNK = QF // NCH         # 16
FW_SPLIT = 44              # fc1 its in the first (big) weight DMA
EPS_BN = 1e-5
# Per-core BN statistics (skip the stats AllGathers).  Each core normalizes
# with moments of its own 64000 points; sampling error ~0.4% which largely
# cancels through the per-segment softmax.  Set False for exact global stats.
LOCAL_STATS = False
F32 = mybir.dt.float32
BF16 = mybir.dt.float16  # fp16: same speed as bf16, 8x lower rounding noise
RG = [list(range(NCORES))]
AF = mybir.ActivationFunctionType
ALU = mybir.AluOpType

_cache = {}


# ------------------------------------------------------------------ walrus fix
def _install_walrus_patch():
    """This container's walrus accepts only ONE semaphore wait per instruction.
    Spread Tile's end-of-kernel drain waits across single-wait nops, and split
    any instruction carrying >1 waits onto same-engine carrier nops."""
    if _cache.get("patched"):
        return
    from concourse.vector_clock import ScopedClock, VectorClock

    counter = [0]

    def split_waits(nc):
        for bb in nc.main_func.blocks:
            out = []
            changed = False
            for ins in bb.instructions:
                si = ins.sync_info
                waits = list(si.on_wait) if si and si.on_wait else []
                if len(waits) > 1:
                    changed = True
                    for w in waits[:-1]:
                        counter[0] += 1
                        out.append(mybir.InstNoOp(
                            name=f"I-wsplit-{counter[0]}",
                            engine=ins.engine, ins=[], outs=[],
                            sync_info=mybir.SyncInfo(on_wait=[w], on_update=[]),
                            bass_nofuse=True))
                    si.on_wait = waits[-1:]
                out.append(ins)
            if changed:
                try:
                    bb.instructions = out
                except Exception:
                    bb.instructions.clear()
                    for x in out:
                        bb.instructions.append(x)

    def _patched(self, tick_clock, wait_clock):
        nc = self.nc
        gc = tick_clock.global_clock
        n = len(gc)
        for i in range(n):
            if gc[i] > 0:
                vec = [0] * n
                vec[i] = gc[i]
                nop = nc.sync.nop(nofuse=True, hint=f"drain_wait_p{i}")
                wait_clock.add_sem_waits(
                    nop.ins, ScopedClock({None: VectorClock(vec)}))
        nc.sync.drain()
        nc.all_engine_barrier()
        assert self.sems is not None
        popped = nc._tile_sem_poison_stack.pop()
        assert popped is self._sem_poison
        nc.clear_and_free_semaphores(list(self.sems.allocated().values()))
        nc.all_engine_barrier()
        split_waits(nc)

    tile.TileContext._drain_and_barrier = _patched
    _cache["patched"] = True


# ------------------------------------------------------------------ bass build
def _build():
    _install_walrus_patch()
    nc = bass.Bass()

    def ein(name, shape, dt):
        return nc.dram_tensor(name, shape, dt, kind="ExternalInput")

    d = {}
    d["xA4"] = ein("xA4", [128, QF], BF16)
    d["xB"] = ein("xB", [PH, C * 2 * B], BF16)
    d["w1D"] = ein("w1D", [128, 128], BF16)
    d["w2D"] = ein("w2D", [128, 128], BF16)
    d["w3sp"] = ein("w3sp", [128, 4], BF16)
    for n in ("g1q", "be1q", "g2q", "be2q"):
        d[n] = ein(n, [128, 1], F32)
    d["g3s"] = ein("g3s", [1, 1], F32)
    d["be3s"] = ein("be3s", [1, 1], F32)
    d["f1"] = ein("f1", [128, 16], F32)
    d["ft1"] = ein("ft1", [16, 128], F32)
    d["f2"] = ein("f2", [128, 8], F32)
    d["ft2"] = ein("ft2", [8, 128], F32)
    d["f8_16"] = ein("f8_16", [128, 16], F32)
    d["f8_8"] = ein("f8_8", [64, 8], F32)
    d["fw1t"] = ein("fw1t", [PH, C * 2 * 1024], BF16)
    d["fw2t"] = ein("fw2t", [128, 256], BF16)
    d["fg1s"] = ein("fg1s", [128, 1], F32)
    d["fbe1s"] = ein("fbe1s", [128, 1], F32)
    d["fg2t"] = ein("fg2t", [128, 2], F32)
    d["fbe2t"] = ein("fbe2t", [128, 2], F32)
    d["out_final"] = nc.dram_tensor("out_final", [256, 256], F32,
                                    kind="ExternalOutput")
    # collective bounce buffers
    d["warm_i"] = nc.dram_tensor("warm_i", [1, 4], F32)
    d["warm_o"] = nc.dram_tensor("warm_o", [8, 4], F32)
    d["st1_i"] = nc.dram_tensor("st1_i", [16, 2], F32)
    d["st1_o"] = nc.dram_tensor("st1_o", [128, 2], F32)
    d["st2_i"] = nc.dram_tensor("st2_i", [8, 2], F32)
    d["st2_o"] = nc.dram_tensor("st2_o", [64, 2], F32)
    d["st3_i"] = nc.dram_tensor("st3_i", [1, 2], F32)
    d["st3_o"] = nc.dram_tensor("st3_o", [8, 2], F32)
    d["rs5_i"] = nc.dram_tensor("rs5_i", [NCORES * 129, 256], BF16)
    d["rs5_o"] = nc.dram_tensor("rs5_o", [129, 256], BF16)
    d["ar6_i"] = nc.dram_tensor("ar6_i", [256, 256], BF16)
    d["ar6_o"] = nc.dram_tensor("ar6_o", [256, 256], BF16)

    with tile.TileContext(nc) as tc:
        _body(nc, tc, d)
    return nc


def _mkstats(nc, pool, mv, count, name):
    """mv [p,2]=(mean,var) -> (sum,sumsq) [p,2]."""
    p = mv.shape[0]
    ss = pool.tile([p, 2], F32, tag=f"ss_{name}")
    nc.vector.tensor_mul(ss[:, 1:2], mv[:, 0:1], mv[:, 0:1])
    nc.vector.tensor_add(ss[:, 1:2], ss[:, 1:2], mv[:, 1:2])
    nc.scalar.mul(ss[:, 0:1], mv[:, 0:1], float(count))
    nc.scalar.mul(ss[:, 1:2], ss[:, 1:2], float(count))
    return ss


def _mv_from_ss(nc, pool, ss, count, name):
    """(sum,sumsq) [p,2] over count -> (mean, rstd) [p,2]."""
    p = ss.shape[0]
    mr = pool.tile([p, 2], F32, tag=f"mr_{name}")
    epst = pool.tile([p, 1], F32, tag=f"eps_{name}")
    nc.vector.memset(epst[:], EPS_BN)
    nc.scalar.mul(mr[:, 0:1], ss[:, 0:1], 1.0 / count)
    nc.scalar.mul(mr[:, 1:2], ss[:, 1:2], 1.0 / count)
    m2 = pool.tile([p, 1], F32, tag=f"m2_{name}")
    nc.vector.tensor_mul(m2[:], mr[:, 0:1], mr[:, 0:1])
    nc.vector.tensor_sub(mr[:, 1:2], mr[:, 1:2], m2[:])
    nc.scalar.activation(mr[:, 1:2], mr[:, 1:2], AF.Sqrt, bias=epst[:])
    nc.vector.reciprocal(mr[:, 1:2], mr[:, 1:2])
    return mr


def _scale_bias(nc, pool, mrq, g, be, name):
    """scale = g*rstd ; bias = be - scale*mean  (all [p,1] per-partition)."""
    p = mrq.shape[0]
    sc = pool.tile([p, 1], F32, tag=f"sc_{name}")
    bi = pool.tile([p, 1], F32, tag=f"bi_{name}")
    nc.vector.tensor_mul(sc[:], g[:], mrq[:, 1:2])
    nc.vector.tensor_mul(bi[:], sc[:], mrq[:, 0:1])
    nc.vector.tensor_sub(bi[:], be[:], bi[:])
    return sc, bi


def _body(nc, tc, d):
    sing_cm = tc.tile_pool(name="sing", bufs=1)
    bigY_cm = tc.tile_pool(name="bigY", bufs=1)   # xb + exp tiles: live to FC1
    work_cm = tc.tile_pool(name="work", bufs=1)
    fwA_cm = tc.tile_pool(name="fwA", bufs=1)
    bigX_cm = tc.tile_pool(name="bigX", bufs=1)   # xa/h2 + h1: dies after L3
    psA_cm = tc.tile_pool(name="psA", bufs=2, space="PSUM")
    psS_cm = tc.tile_pool(name="psS", bufs=1, space="PSUM")
    sing = sing_cm.__enter__()
    bigY = bigY_cm.__enter__()
    work = work_cm.__enter__()
    fwA_p = fwA_cm.__enter__()
    bigX = bigX_cm.__enter__()
    psA = psA_cm.__enter__(); psS = psS_cm.__enter__()

    # ---------------- big loads FIRST: the HWDGE issue path drains roughly
    # in issue order, so xa (gates L1) and xb go before everything else.
    xa = bigX.tile([128, QF], BF16, tag="slotA")      # xa -> (dead) -> h2
    nc.sync.dma_start(xa[:], d["xA4"][:])
    xb = bigY.tile([PH, C * 2 * B], BF16, tag="xb")
    nc.scalar.dma_start(xb[:], d["xB"][:])
    xbv = xb[:].rearrange("p (c h a s) -> p c h a s", c=C, h=2, a=4, s=64)

    # ---------------- constants (sync ring; small)
    def load(name, shape, dt=F32, pool=sing):
        t = pool.tile(shape, dt, tag=name)
        nc.sync.dma_start(t[:], d[name][:])
        return t

    w1D = load("w1D", [128, 128], BF16)
    w2D = load("w2D", [128, 128], BF16)
    w3sp = load("w3sp", [128, 4], BF16)
    f1s = load("f1", [128, 16])
    ft1s = load("ft1", [16, 128])
    f2s = load("f2", [128, 8])
    ft2s = load("ft2", [8, 128])
    f8_16s = load("f8_16", [128, 16])
    f8_8s = load("f8_8", [64, 8])
    g1 = load("g1q", [128, 1]); be1 = load("be1q", [128, 1])
    g2 = load("g2q", [128, 1]); be2 = load("be2q", [128, 1])
    g3 = load("g3s", [1, 1]); be3 = load("be3s", [1, 1])
    ones128 = sing.tile([128, 1], F32)
    nc.vector.memset(ones128[:], 1.0)
    ones8 = sing.tile([8, 1], F32)
    nc.vector.memset(ones8[:], 1.0)
    ones125 = sing.tile([PH, 1], F32)
    nc.vector.memset(ones125[:], 1.0)
    ones1x = sing.tile([1, 128], F32)
    nc.vector.memset(ones1x[:], 1.0)
    ident = sing.tile([128, 128], F32)
    make_identity(nc, ident[:])
    ident16 = sing.tile([128, 128], BF16)
    make_identity(nc, ident16[:])

    # fw1 head: chunked 8-its per InstDMACopy on alternating rings, queued
    # behind xa/xb/consts.
    fwv = d["fw1t"][:].rearrange("p (i o) -> p i o", i=C * 2, o=1024)
    fwAt = fwA_p.tile([PH, FW_SPLIT, 1024], BF16, tag="fwA")
    for k in range(0, FW_SPLIT, 8):
        hi = min(k + 8, FW_SPLIT)
        nc.sync.dma_start(fwAt[:, k:hi, :], fwv[:, k:hi, :])


    def stage_layer(rhs_src, wD, fold, foldT, f8fold, st_i, st_o,
                    gq, beq, name, out_tag):
        """Quartered A-orientation layer: matmuls -> raw evict (scalar) +
        bn_stats (vector, from PSUM), fold + AllGather stats, then BN+relu
        applied in place, split scalar/vector."""
        y = bigX.tile([128, QF], BF16, tag=out_tag, name=f"y_{name}")
        stat = work.tile([128, 2 * NCHUNK, 6], F32, tag=f"stat_{name}")
        for j in range(NCHUNK):
            ps = psA.tile([128, 1024], F32, tag="psA", name=f"ps_{name}_{j}")
            base = j * NCH
            nc.tensor.matmul(ps[:, 0:500], wD[:], rhs_src[:, base:base + 500],
                             start=True, stop=True)
            nc.tensor.matmul(ps[:, 512:1012], wD[:],
                             rhs_src[:, base + 500:base + 1000],
                             start=True, stop=True)
            pv = ps[:].rearrange("p (k c) -> p k c", k=2, c=512)[:, :, 0:500]
            nc.scalar.copy(
                y[:, base:base + NCH].rearrange("p (k c) -> p k c", k=2,
                                                c=500), pv)
            nc.vector.bn_stats(stat[:, 2 * j, :], ps[:, 0:500])
            nc.vector.bn_stats(stat[:, 2 * j + 1, :], ps[:, 512:1012])
        mv = work.tile([128, 2], F32, tag=f"mv_{name}")
        nc.vector.bn_aggr(mv[:], stat[:])
        ss = _mkstats(nc, work, mv, QF, name)
        nfold = fold.shape[1]
        psf = psS.tile([128, 2], F32, tag="small", name=f"psf_{name}")
        nc.tensor.matmul(psf[:nfold, :], fold[:], ss[:], start=True, stop=True)
        sbf = work.tile([nfold, 2], F32, tag=f"sbf_{name}")
        nc.scalar.copy(sbf[:], psf[:nfold, :])
        if LOCAL_STATS:
            mr = _mv_from_ss(nc, work, sbf, B * P // NCORES, name)
        else:
            nc.gpsimd.dma_start(st_i[:], sbf[:])
            nc.gpsimd.collective_compute(
                "AllGather", ALU.bypass, replica_groups=RG,
                ins=[st_i[:]], outs=[st_o[:]])
            agg = work.tile([nfold * NCORES, 2], F32, tag=f"agg_{name}")
            nc.gpsimd.dma_start(agg[:], st_o[:])
            psg = psS.tile([128, 2], F32, tag="small", name=f"psg_{name}")
            nc.tensor.matmul(psg[:nfold, :], f8fold[:], agg[:], start=True,
                             stop=True)
            ssg = work.tile([nfold, 2], F32, tag=f"ssg_{name}")
            nc.scalar.copy(ssg[:], psg[:nfold, :])
            mr = _mv_from_ss(nc, work, ssg, B * P, name)
        psb = psS.tile([128, 2], F32, tag="small", name=f"psb_{name}")
        nc.tensor.matmul(psb[:], foldT[:], mr[:], start=True, stop=True)
        mrq = work.tile([128, 2], F32, tag=f"mrq_{name}")
        nc.scalar.copy(mrq[:], psb[:])
        sc, bi = _scale_bias(nc, work, mrq, gq, beq, name)
        # relu in place: scalar takes the first chunks, vector the rest
        NSC = 6
        for j in range(NSC):
            sl = slice(j * NCH, (j + 1) * NCH)
            nc.scalar.activation(y[:, sl], y[:, sl], AF.Relu,
                                 bias=bi[:], scale=sc[:])
        for j in range(NSC, NCHUNK):
            sl = slice(j * NCH, (j + 1) * NCH)
            nc.vector.tensor_scalar(y[:, sl], y[:, sl], sc[:], bi[:],
                                    ALU.mult, ALU.add)
            nc.vector.tensor_scalar_max(y[:, sl], y[:, sl], 0.0)
        return y

    # ---------------- stage A layers 1 & 2
    h1 = stage_layer(xa, w1D, f1s, ft1s, f8_16s,
                     d["st1_i"], d["st1_o"], g1, be1, "l1", "slotB")
    # h2 reuses slot A (xa dead after L1 matmuls)
    h2 = stage_layer(h1, w2D, f2s, ft2s, f8_8s,
                     d["st2_i"], d["st2_o"], g2, be2, "l2", "slotA")

    # ---------------- layer 3 via transpose-matmuls: scores point-major.
    # lhsT = h2[:, 125c:125c+125] (K=128 channel-partitions, M=125 points),
    # rhs = w3 spread [128, 4] (col a = w3 in quarter-a rows) ->
    # psL3[:, 4c+a] = score of quarter a's point 125c+p.
    # Free-dim layout: c = (s, h) with s in 0..63, h in 0..1; col = 8s+4h+a.
    psL3_cm = tc.tile_pool(name="psL3", bufs=1, space="PSUM")
    psL3 = psL3_cm.__enter__()
    l3ps = psL3.tile([PH, 512], F32, tag="l3ps")
    for cgrp in range(128):
        nc.tensor.matmul(l3ps[:, 4 * cgrp: 4 * cgrp + 4],
                         h2[:, 125 * cgrp: 125 * cgrp + 125],
                         w3sp[:], start=True, stop=True)
    # BN3 stats over all points (125*512 = 64000 local)
    stat3 = work.tile([PH, 6], F32, tag="stat3")
    nc.vector.bn_stats(stat3[:], l3ps[:])
    mv3 = work.tile([PH, 2], F32, tag="mv3")
    nc.vector.bn_aggr(mv3[:], stat3[:])
    ss3 = _mkstats(nc, work, mv3, 512, "l3")
    psf3 = psS.tile([128, 2], F32, tag="small", name="psf3")
    nc.tensor.matmul(psf3[:1, :], ones125[:], ss3[:], start=True, stop=True)
    sbf3 = work.tile([1, 2], F32, tag="sbf3")
    nc.scalar.copy(sbf3[:], psf3[:1, :])
    if LOCAL_STATS:
        mr3 = _mv_from_ss(nc, work, sbf3, B * P // NCORES, "l3")
    else:
        nc.gpsimd.dma_start(d["st3_i"][:], sbf3[:])
        nc.gpsimd.collective_compute(
            "AllGather", ALU.bypass, replica_groups=RG,
            ins=[d["st3_i"][:]], outs=[d["st3_o"][:]])
        agg3 = work.tile([8, 2], F32, tag="agg3")
        nc.gpsimd.dma_start(agg3[:], d["st3_o"][:])
        psg3 = psS.tile([128, 2], F32, tag="small", name="psg3")
        nc.tensor.matmul(psg3[:1, :], ones8[:], agg3[:], start=True, stop=True)
        ssg3 = work.tile([1, 2], F32, tag="ssg3")
        nc.scalar.copy(ssg3[:], psg3[:1, :])
        mr3 = _mv_from_ss(nc, work, ssg3, B * P, "l3")
    scb1 = work.tile([1, 2], F32, tag="scb1")
    nc.vector.tensor_mul(scb1[:, 0:1], g3[:], mr3[:, 1:2])
    nc.vector.tensor_mul(scb1[:, 1:2], scb1[:, 0:1], mr3[:, 0:1])
    nc.vector.tensor_sub(scb1[:, 1:2], be3[:], scb1[:, 1:2])
    psb3 = psS.tile([128, 2], F32, tag="small", name="psb3")
    nc.tensor.matmul(psb3[:PH, :], ones1x[:, :PH], scb1[:], start=True,
                     stop=True)
    scb = work.tile([PH, 2], F32, tag="scb")
    nc.scalar.copy(scb[:], psb3[:PH, :])
    # relu(BN3) in place on PSUM, then exp -> attention numerators
    nc.scalar.activation(l3ps[:], l3ps[:], AF.Relu,
                         bias=scb[:, 1:2], scale=scb[:, 0:1])
    expP = bigY.tile([PH, 512], BF16, tag="expP")
    nc.scalar.activation(expP[:], l3ps[:], AF.Exp)
    expv = expP[:].rearrange("p (s h a) -> p h a s", s=64, h=2, a=4)
    # partial softmax denominators: sum over h (vector) then partitions (PE)
    zpart = work.tile([PH, 256], F32, tag="zpart")
    zpv = zpart[:].rearrange("p (a s) -> p a s", a=4, s=64)
    nc.vector.tensor_add(zpv, expv[:, 0], expv[:, 1])
    psz = psS.tile([128, 256], F32, tag="psz")
    nc.tensor.matmul(psz[:1, :], ones125[:], zpart[:], start=True, stop=True)
    zq = work.tile([1, 256], BF16, tag="zq")
    nc.scalar.copy(zq[:], psz[:1, :])
    # z into every shard's aux row of rs5_i (cols already in segment order)
    for cc in range(NCORES):
        nc.sync.dma_start(d["rs5_i"][cc * 129 + 128: cc * 129 + 129, :],
                          zq[:])

    psL3_cm.__exit__(None, None, None)
    psS_cm.__exit__(None, None, None)
    psA_cm.__exit__(None, None, None)
    bigX_cm.__exit__(None, None, None)

    # xa/h1/h2 are dead now -> their SBUF region hosts the fw1 tail chunk
    # and the small tail tiles.
    big2_cm = tc.tile_pool(name="big2", bufs=1)
    big2 = big2_cm.__enter__()
    fwBt = big2.tile([PH, C * 2 - FW_SPLIT, 1024], BF16, tag="fwB")
    for k in range(0, C * 2 - FW_SPLIT, 8):
        hi = min(k + 8, C * 2 - FW_SPLIT)
        nc.sync.dma_start(fwBt[:, k:hi, :],
                          fwv[:, FW_SPLIT + k:FW_SPLIT + hi, :])

    # ---------------- FC1 (contraction-sharded, out [1024, 256] partial)
    psF_cm = tc.tile_pool(name="psF", bufs=1, space="PSUM")
    ptp_cm = tc.tile_pool(name="ptp", bufs=3)
    psF = psF_cm.__enter__()
    ptp = ptp_cm.__enter__()
    # pt-stationary "swap" arrangement: lhsT = a 128-segment half of pt,
    # rhs = the full 1024-wide fw row (fp16 moving operand) -> out is
    # [seg-half, fc1out] in PSUM; 2 matmuls per it instead of 8 (each
    # InstMatmult carries a fused LDWEIGHTS here, so fewer/wider wins).
    r1ps = [psF.tile([128, 1024], F32, name=f"r1ps_{hh}", tag=f"r1_{hh}")
            for hh in range(2)]
    NIT = C * 2
    for ch in range(C):
        for h in range(2):
            it = ch * 2 + h
            fw = (fwAt[:, it, :] if it < FW_SPLIT
                  else fwBt[:, it - FW_SPLIT, :])
            pt = ptp.tile([PH, 256], BF16, tag="pt", name=f"pt_{it}")
            ptv = pt[:].rearrange("p (a s) -> p a s", a=4, s=64)
            nc.vector.tensor_mul(ptv, xbv[:, ch, h], expv[:, h])
            for hh in range(2):
                for q in range(2):
                    nc.tensor.matmul(
                        r1ps[hh][:, q * 512: q * 512 + 512],
                        pt[:, hh * 128: hh * 128 + 128],
                        fw[:, q * 512: q * 512 + 512],
                        start=(it == 0), stop=(it == NIT - 1))
    # transpose [seg, out] -> [out, seg] before staging (RS shards are
    # out-major).  Evict each half to fp16, then 16 PE transposes.
    # per-block evict -> PE transpose -> stage, pipelined across scalar /
    # vector / tensor; everything fits in psF (r1ps 4 banks + fc1T 2).
    _rings = [nc.sync, nc.sync]
    r1fl = [None, None]
    for hh in range(2):
        r1fl[hh] = big2.tile([128, 1024], BF16, tag=f"r1fl_{hh}",
                             name=f"r1fl_{hh}")
    for m in range(8):
        for hh in range(2):
            blk = slice(m * 128, m * 128 + 128)
            if (m + hh) % 2 == 0:
                nc.scalar.copy(r1fl[hh][:, blk], r1ps[hh][:, blk])
            else:
                nc.vector.tensor_copy(r1fl[hh][:, blk], r1ps[hh][:, blk])
    for m in range(8):
        r1sb = big2.tile([128, 256], BF16, tag="r1sb", name=f"r1sb_{m}",
                         bufs=2)
        for hh in range(2):
            ps_t = psF.tile([128, 128], BF16, tag="fc1T",
                            name=f"fc1T_{m}_{hh}", bufs=2)
            nc.tensor.transpose(ps_t[:], r1fl[hh][:, m * 128: m * 128 + 128],
                                ident16[:])
            nc.scalar.copy(r1sb[:, hh * 128: hh * 128 + 128], ps_t[:])
        _rings[m % 2].dma_start(d["rs5_i"][m * 129: m * 129 + 128, :],
                                r1sb[:])
    nc.gpsimd.collective_compute(
        "ReduceScatter", ALU.add, replica_groups=RG,
        ins=[d["rs5_i"][:]], outs=[d["rs5_o"][:]])

    ptp_cm.__exit__(None, None, None)
    psF_cm.__exit__(None, None, None)

    # ---------------- FC1 finish + FC2 + tail
    ps2_cm = tc.tile_pool(name="ps2", bufs=1, space="PSUM")
    ps2 = ps2_cm.__enter__()

    r1h = big2.tile([128, 256], BF16, tag="r1h")
    nc.sync.dma_start(r1h[:], d["rs5_o"][0:128, :])
    zrow = work.tile([1, 256], BF16, tag="zrow")
    nc.scalar.dma_start(zrow[:], d["rs5_o"][128:129, :])
    zrec = work.tile([1, 256], F32, tag="zrec")
    nc.vector.reciprocal(zrec[:], zrow[:])
    ps_z = ps2.tile([128, 256], F32, tag="nrmb", name="zb")
    nc.tensor.matmul(ps_z[:], ones1x[:], zrec[:], start=True, stop=True)
    zinv = big2.tile([128, 256], BF16, tag="zinv")
    nc.scalar.copy(zinv[:], ps_z[:])
    r1 = big2.tile([128, 256], F32, tag="r1")
    nc.vector.tensor_mul(r1[:], r1h[:], zinv[:])
    # BN over segments (free dim), relu
    stf1 = work.tile([128, 6], F32, tag="stf1")
    nc.vector.bn_stats(stf1[:], r1[:])
    mvf1 = work.tile([128, 2], F32, tag="mvf1")
    nc.vector.bn_aggr(mvf1[:], stf1[:])
    epsf = work.tile([128, 1], F32, tag="epsf")
    nc.vector.memset(epsf[:], EPS_BN)
    nc.scalar.activation(mvf1[:, 1:2], mvf1[:, 1:2], AF.Sqrt, bias=epsf[:])
    nc.vector.reciprocal(mvf1[:, 1:2], mvf1[:, 1:2])
    fg1 = load("fg1s", [128, 1], pool=work)
    fbe1 = load("fbe1s", [128, 1], pool=work)
    scf1, bif1 = _scale_bias(nc, work, mvf1, fg1, fbe1, "f1")
    r1b = big2.tile([128, 256], BF16, tag="r1b")
    nc.scalar.activation(r1b[:], r1[:], AF.Relu, bias=bif1[:], scale=scf1[:])
    # FC2 partial
    fw2 = load("fw2t", [128, 256], BF16, pool=work)
    r2sb = big2.tile([128, 2, 256], BF16, tag="r2sb")
    for m in range(2):
        ps_r2 = ps2.tile([128, 256], F32, tag=f"r2_{m}")
        nc.tensor.matmul(ps_r2[:], fw2[:, m * 128: (m + 1) * 128], r1b[:],
                         start=True, stop=True)
        nc.scalar.copy(r2sb[:, m, :], ps_r2[:])
        _rings[m].dma_start(d["ar6_i"][m * 128: (m + 1) * 128, :],
                            r2sb[:, m, :])
    nc.gpsimd.collective_compute(
        "AllReduce", ALU.add, replica_groups=RG,
        ins=[d["ar6_i"][:]], outs=[d["ar6_o"][:]])

    # tail: BN over segments per o2-row + relu (both blocks), L2-norm via
    # ones-matmul row-sums of squares (before the transposes), then transpose.
    fg2 = load("fg2t", [128, 2], pool=work)
    fbe2 = load("fbe2t", [128, 2], pool=work)
    r2s = []
    ps_n = ps2.tile([128, 256], F32, tag="nrm")
    for m in range(2):
        r2 = big2.tile([128, 256], BF16, tag=f"r2_{m}", name=f"r2_{m}")
        _rings[m].dma_start(r2[:], d["ar6_o"][m * 128: (m + 1) * 128, :])
        stf2 = work.tile([128, 6], F32, tag=f"stf2_{m}")
        nc.vector.bn_stats(stf2[:], r2[:])
        mvf2 = work.tile([128, 2], F32, tag=f"mvf2_{m}")
        nc.vector.bn_aggr(mvf2[:], stf2[:])
        nc.scalar.activation(mvf2[:, 1:2], mvf2[:, 1:2], AF.Sqrt, bias=epsf[:])
        nc.vector.reciprocal(mvf2[:, 1:2], mvf2[:, 1:2])
        scf2, bif2 = _scale_bias(nc, work, mvf2,
                                 fg2[:, m: m + 1], fbe2[:, m: m + 1],
                                 f"f2_{m}")
        nc.scalar.activation(r2[:], r2[:], AF.Relu, bias=bif2[:], scale=scf2[:])
        sq = big2.tile([128, 256], F32, tag=f"sq_{m}", name=f"sq_{m}")
        nc.scalar.activation(sq[:], r2[:], AF.Square)
        nc.tensor.matmul(ps_n[:1, :], ones128[:], sq[:],
                         start=(m == 0), stop=(m == 1))
        r2s.append(r2)
    nrm = work.tile([1, 256], F32, tag="nrmrow")
    nc.scalar.activation(nrm[:], ps_n[:1, :], AF.Sqrt)
    nc.vector.tensor_scalar_max(nrm[:], nrm[:], 1e-12)
    nc.vector.reciprocal(nrm[:], nrm[:])
    ps_nb = ps2.tile([128, 256], F32, tag="nrmb")
    nc.tensor.matmul(ps_nb[:], ones1x[:], nrm[:], start=True, stop=True)
    nrmb = big2.tile([128, 256], BF16, tag="nrmbs")
    nc.scalar.copy(nrmb[:], ps_nb[:])
    outT = big2.tile([128, 2, 256], F32, tag="outT")
    for m in range(2):
        nc.vector.tensor_mul(r2s[m][:], r2s[m][:], nrmb[:])
        for tt in range(2):
            ps_t = ps2.tile([128, 128], BF16, tag="tailT",
                            name=f"tailT_{m}_{tt}", bufs=2)
            nc.tensor.transpose(ps_t[:], r2s[m][:, tt * 128: (tt + 1) * 128],
                                ident16[:])
            nc.scalar.copy(outT[:, tt, m * 128: (m + 1) * 128], ps_t[:])
    for tt in range(2):
        _rings[tt].dma_start(d["out_final"][tt * 128: (tt + 1) * 128, :],
                             outT[:, tt, :])

    ps2_cm.__exit__(None, None, None)
    big2_cm.__exit__(None, None, None)
    fwA_cm.__exit__(None, None, None)
    work_cm.__exit__(None, None, None)
    bigY_cm.__exit__(None, None, None)
    sing_cm.__exit__(None, None, None)


# ------------------------------------------------------------------ host side
def _prep_core(x3, fw1, c):
    xs = x3[:, PL * c: PL * (c + 1), :]                        # [256,250,32]
    arr = np.ascontiguousarray(xs.transpose(2, 0, 1))          # [32,256,250]
    xA4 = arr.reshape(C, 4, QF).transpose(1, 0, 2).reshape(128, QF)
    xb = xs.reshape(B, 2, PH, C).transpose(2, 3, 1, 0)         # [125,32,2,256]
    xB = np.ascontiguousarray(xb).reshape(PH, C * 2 * B)
    fw = fw1.reshape(1024, P, C)[:, PL * c: PL * (c + 1), :]
    fw = fw.reshape(1024, 2, PH, C).transpose(2, 3, 1, 0)      # [125,32,2,1024]
    fw1t = np.ascontiguousarray(fw).reshape(PH, C * 2 * 1024)
    bf = np.float16
    return (np.ascontiguousarray(xA4).astype(bf), xB.astype(bf),
            fw1t.astype(bf))


def _qrep(v, rows):
    out = np.zeros((128, 1), np.float32)
    for a in range(4):
        out[32 * a: 32 * a + rows, 0] = v
    return out


def _wdiag(w):
    """w [out,in] -> block-diagonal lhsT [128, 128]: block a (32x32) holds
    w.T in its top-left corner."""
    t = np.zeros((128, 128), np.float32)
    wt = w.T  # [in, out]
    for a in range(4):
        t[32 * a: 32 * a + wt.shape[0], 32 * a: 32 * a + wt.shape[1]] = wt
    return t


def kernel(**inputs):
    if "nc" not in _cache:
        _cache["nc"] = _build()
    nc = _cache["nc"]
    bf = np.float16

    g = {k: np.asarray(v, np.float32) for k, v in inputs.items()
         if k != "length"}
    x3 = g["x"].reshape(B, P, C)

    f1 = np.zeros((128, 16), np.float32)
    f2 = np.zeros((128, 8), np.float32)
    for a in range(4):
        f1[32 * a: 32 * a + 16, :] = np.eye(16, dtype=np.float32)
        f2[32 * a: 32 * a + 8, :] = np.eye(8, dtype=np.float32)
    f8_16 = np.zeros((128, 16), np.float32)
    f8_8 = np.zeros((64, 8), np.float32)
    for k in range(8):
        f8_16[16 * k: 16 * k + 16, :] = np.eye(16, dtype=np.float32)
        f8_8[8 * k: 8 * k + 8, :] = np.eye(8, dtype=np.float32)
    w3sp = np.zeros((128, 4), np.float32)
    for a in range(4):
        w3sp[32 * a: 32 * a + 8, a] = g["w3"][0, :]

    shared = {
        "w1D": _wdiag(g["w1"]).astype(bf),
        "w2D": _wdiag(g["w2"]).astype(bf),
        "w3sp": w3sp.astype(bf),
        "g1q": _qrep(g["g1"], 16), "be1q": _qrep(g["be1"], 16),
        "g2q": _qrep(g["g2"], 8), "be2q": _qrep(g["be2"], 8),
        "g3s": g["g3"].reshape(1, 1), "be3s": g["be3"].reshape(1, 1),
        "f1": f1, "ft1": np.ascontiguousarray(f1.T),
        "f2": f2, "ft2": np.ascontiguousarray(f2.T),
        "f8_16": f8_16, "f8_8": f8_8,
        "fg2t": np.ascontiguousarray(g["fg2"].reshape(2, 128).T),
        "fbe2t": np.ascontiguousarray(g["fbe2"].reshape(2, 128).T),
    }

    in_maps = []
    for c in range(NCORES):
        xA4, xB, fw1t = _prep_core(x3, g["fw1"], c)
        m = dict(shared)
        m["xA4"] = xA4
        m["xB"] = xB
        m["fw1t"] = fw1t
        m["fw2t"] = np.ascontiguousarray(
            g["fw2"][:, 128 * c: 128 * (c + 1)].T).astype(bf)
        m["fg1s"] = g["fg1"][128 * c: 128 * (c + 1)].reshape(128, 1)
        m["fbe1s"] = g["fbe1"][128 * c: 128 * (c + 1)].reshape(128, 1)
        in_maps.append(m)

    from concourse.bass_utils import run_bass_kernel_spmd

    res = run_bass_kernel_spmd(nc, in_maps, core_ids=list(range(NCORES)),
                               trace=bool(_cache.get("trace")))
    _cache["last_result"] = res
    return np.asarray(res.results[0]["out_final"], np.float32)


if __name__ == "__main__":
    nc = _build()
    print("build ok; instructions:",
          sum(len(bb.instructions) for bb in nc.main_func.blocks))


# revision 38
# speedup vs baseline: 1.2028x; 1.0586x over previous
"""Trainium2 Bass kernel for nn_FCGF_point_att3_sft_7000 (8 NeuronCores).

Model: pointwise attention MLP (32->16->8->1, BN+relu, BN stats over the full
512000-point batch), per-segment softmax over 2000 points, attention-weighted
pooling to [256, 64000], FC head 64000->1024->256 (BN+relu, stats over the
256-segment batch), final L2 row-normalize.

Sharding: points-within-segment. Core c owns points p in [250c, 250(c+1)) of
every segment. Stage A is data-parallel over points with AllGather'd BN stats;
fc1 is contraction-sharded (each core owns 8000 of the 64000 inputs and the
matching fw1 rows), summed via ReduceScatter whose per-shard aux row also
carries the softmax denominators; fc2 is contraction-sharded and finished with
an AllReduce; the tail is replicated.

Layout notes:
- Stage A layers 1-2 run in "quartered" A-orientation: x.T is [128, 16000]
  with the channels of free-quarter a on partitions [32a, 32a+32), matmuls
  use a block-diagonal lhsT so every chunk op runs 128 partitions wide.
- Layer 3 uses transpose-matmuls: lhsT = h2 point-chunk [128, 125], rhs =
  w3 spread over the 4 quarter blocks [128, 4] -> scores land point-major in
  a single PSUM bank [125, (seg64, half2, quarter4)].  relu/exp/softmax-z all
  run on that tiny tile; the old y3 eviction / repack / PE-transposes vanish.
- A zero-dependency warmup AllGather triggers at t~0 so the ~50us ncfw init
  overlaps the input DMAs and stage-A layer 1.
- fw1 streams as two big contiguous DMAs (96KB + 32KB per partition runs).
- ReduceScatter runs in fp16 (partials + softmax z), AllReduce in f32.

Training-mode BN is shift-invariant => conv/linear biases (b1,b2,b3,fb1,fb2)
drop out exactly; they are accepted and ignored.
"""

import sys

sys.path.insert(0, "/opt/trn_rl_repo")

import numpy as np

import concourse.bass as bass
import concourse.tile as tile
from concourse import mybir
from concourse.masks import make_identity

B = 256
P = 2000
C = 32
NCORES = 8
PL = P // NCORES           # 250
PH = PL // 2               # 125
NPTS = B * PL              # 64000 points per core
QF = NPTS // 4             # 16000 per quarter
NCH = 1000                 # stage-A eviction chunk (two 500-col matmuls)
NCHUNK = QF // NCH         # 16
FW_SPLIT = 44              # fc1 its in the first (big) weight DMA
EPS_BN = 1e-5
# Per-core BN statistics (skip the stats AllGathers).  Each core normalizes
# with moments of its own 64000 points; sampling error ~0.4% which largely
# cancels through the per-segment softmax.  Set False for exact global stats.
LOCAL_STATS = False
F32 = mybir.dt.float32
BF16 = mybir.dt.float16  # fp16: same speed as bf16, 8x lower rounding noise
RG = [list(range(NCORES))]
AF = mybir.ActivationFunctionType
ALU = mybir.AluOpType

_cache = {}


# ------------------------------------------------------------------ walrus fix
def _install_walrus_patch():
    """This container's walrus accepts only ONE semaphore wait per instruction.
    Spread Tile's end-of-kernel drain waits across single-wait nops, and split
    any instruction carrying >1 waits onto same-engine carrier nops."""
    if _cache.get("patched"):
        return
    from concourse.vector_clock import ScopedClock, VectorClock

    counter = [0]

    def split_waits(nc):
        for bb in nc.main_func.blocks:
            out = []
            changed = False
            for ins in bb.instructions:
                si = ins.sync_info
                waits = list(si.on_wait) if si and si.on_wait else []
                if len(waits) > 1:
                    changed = True
                    for w in waits[:-1]:
                        counter[0] += 1
                        out.append(mybir.InstNoOp(
                            name=f"I-wsplit-{counter[0]}",
                            engine=ins.engine, ins=[], outs=[],
                            sync_info=mybir.SyncInfo(on_wait=[w], on_update=[]),
                            bass_nofuse=True))
                    si.on_wait = waits[-1:]
                out.append(ins)
            if changed:
                try:
                    bb.instructions = out
                except Exception:
                    bb.instructions.clear()
                    for x in out:
                        bb.instructions.append(x)

    def _patched(self, tick_clock, wait_clock):
        nc = self.nc
        gc = tick_clock.global_clock
        n = len(gc)
        for i in range(n):
            if gc[i] > 0:
                vec = [0] * n
                vec[i] = gc[i]
                nop = nc.sync.nop(nofuse=True, hint=f"drain_wait_p{i}")
                wait_clock.add_sem_waits(
                    nop.ins, ScopedClock({None: VectorClock(vec)}))
        nc.sync.drain()
        nc.all_engine_barrier()
        assert self.sems is not None
        popped = nc._tile_sem_poison_stack.pop()
        assert popped is self._sem_poison
        nc.clear_and_free_semaphores(list(self.sems.allocated().values()))
        nc.all_engine_barrier()
        split_waits(nc)

    tile.TileContext._drain_and_barrier = _patched
    _cache["patched"] = True


# ------------------------------------------------------------------ bass build
def _build():
    _install_walrus_patch()
    nc = bass.Bass()

    def ein(name, shape, dt):
        return nc.dram_tensor(name, shape, dt, kind="ExternalInput")

    d = {}
    d["xA4"] = ein("xA4", [128, QF], BF16)
    d["xB"] = ein("xB", [PH, C * 2 * B], BF16)
    d["w1D"] = ein("w1D", [128, 128], BF16)
    d["w2D"] = ein("w2D", [128, 128], BF16)
    d["w3sp"] = ein("w3sp", [128, 4], BF16)
    for n in ("g1q", "be1q", "g2q", "be2q"):
        d[n] = ein(n, [128, 1], F32)
    d["g3s"] = ein("g3s", [1, 1], F32)
    d["be3s"] = ein("be3s", [1, 1], F32)
    d["f1"] = ein("f1", [128, 16], F32)
    d["ft1"] = ein("ft1", [16, 128], F32)
    d["f2"] = ein("f2", [128, 8], F32)
    d["ft2"] = ein("ft2", [8, 128], F32)
    d["f8_16"] = ein("f8_16", [128, 16], F32)
    d["f8_8"] = ein("f8_8", [64, 8], F32)
    d["fw1t"] = ein("fw1t", [PH, C * 2 * 1024], BF16)
    d["fw2t"] = ein("fw2t", [128, 256], BF16)
    d["fg1s"] = ein("fg1s", [128, 1], F32)
    d["fbe1s"] = ein("fbe1s", [128, 1], F32)
    d["fg2t"] = ein("fg2t", [128, 2], F32)
    d["fbe2t"] = ein("fbe2t", [128, 2], F32)
    d["out_final"] = nc.dram_tensor("out_final", [256, 256], F32,
                                    kind="ExternalOutput")
    # collective bounce buffers
    d["warm_i"] = nc.dram_tensor("warm_i", [1, 4], F32)
    d["warm_o"] = nc.dram_tensor("warm_o", [8, 4], F32)
    d["st1_i"] = nc.dram_tensor("st1_i", [16, 2], F32)
    d["st1_o"] = nc.dram_tensor("st1_o", [128, 2], F32)
    d["st2_i"] = nc.dram_tensor("st2_i", [8, 2], F32)
    d["st2_o"] = nc.dram_tensor("st2_o", [64, 2], F32)
    d["st3_i"] = nc.dram_tensor("st3_i", [1, 2], F32)
    d["st3_o"] = nc.dram_tensor("st3_o", [8, 2], F32)
    d["rs5_i"] = nc.dram_tensor("rs5_i", [NCORES * 129, 256], BF16)
    d["rs5_o"] = nc.dram_tensor("rs5_o", [129, 256], BF16)
    d["ar6_i"] = nc.dram_tensor("ar6_i", [256, 256], BF16)
    d["ar6_o"] = nc.dram_tensor("ar6_o", [256, 256], BF16)

    with tile.TileContext(nc) as tc:
        _body(nc, tc, d)
    return nc


def _mkstats(nc, pool, mv, count, name):
    """mv [p,2]=(mean,var) -> (sum,sumsq) [p,2].  All on vector to avoid
    cross-engine hops on the stats critical path."""
    p = mv.shape[0]
    ss = pool.tile([p, 2], F32, tag=f"ss_{name}")
    nc.vector.tensor_mul(ss[:, 1:2], mv[:, 0:1], mv[:, 0:1])
    nc.vector.tensor_add(ss[:, 1:2], ss[:, 1:2], mv[:, 1:2])
    nc.vector.tensor_scalar_mul(ss[:, 0:1], mv[:, 0:1], float(count))
    nc.vector.tensor_scalar_mul(ss[:, 1:2], ss[:, 1:2], float(count))
    return ss


def _mv_from_ss(nc, pool, ss, count, name):
    """(sum,sumsq) [p,2] over count -> (mean, rstd) [p,2].  Vector-only
    except the Sqrt (scalar LUT)."""
    p = ss.shape[0]
    mr = pool.tile([p, 2], F32, tag=f"mr_{name}")
    epst = pool.tile([p, 1], F32, tag=f"eps_{name}")
    nc.vector.memset(epst[:], EPS_BN)
    nc.vector.tensor_scalar_mul(mr[:], ss[:], 1.0 / count)
    m2 = pool.tile([p, 1], F32, tag=f"m2_{name}")
    nc.vector.tensor_mul(m2[:], mr[:, 0:1], mr[:, 0:1])
    nc.vector.tensor_sub(mr[:, 1:2], mr[:, 1:2], m2[:])
    nc.scalar.activation(mr[:, 1:2], mr[:, 1:2], AF.Sqrt, bias=epst[:])
    nc.vector.reciprocal(mr[:, 1:2], mr[:, 1:2])
    return mr


def _scale_bias(nc, pool, mrq, g, be, name):
    """scale = g*rstd ; bias = be - scale*mean  (all [p,1] per-partition)."""
    p = mrq.shape[0]
    sc = pool.tile([p, 1], F32, tag=f"sc_{name}")
    bi = pool.tile([p, 1], F32, tag=f"bi_{name}")
    nc.vector.tensor_mul(sc[:], g[:], mrq[:, 1:2])
    nc.vector.tensor_mul(bi[:], sc[:], mrq[:, 0:1])
    nc.vector.tensor_sub(bi[:], be[:], bi[:])
    return sc, bi


def _body(nc, tc, d):
    sing_cm = tc.tile_pool(name="sing", bufs=1)
    bigY_cm = tc.tile_pool(name="bigY", bufs=1)   # xb + exp tiles: live to FC1
    work_cm = tc.tile_pool(name="work", bufs=1)
    fwA_cm = tc.tile_pool(name="fwA", bufs=1)
    bigX_cm = tc.tile_pool(name="bigX", bufs=1)   # xa/h2 + h1: dies after L3
    psA_cm = tc.tile_pool(name="psA", bufs=2, space="PSUM")
    psS_cm = tc.tile_pool(name="psS", bufs=1, space="PSUM")
    sing = sing_cm.__enter__()
    bigY = bigY_cm.__enter__()
    work = work_cm.__enter__()
    fwA_p = fwA_cm.__enter__()
    bigX = bigX_cm.__enter__()
    psA = psA_cm.__enter__(); psS = psS_cm.__enter__()

    # ---------------- big loads FIRST: the HWDGE issue path drains roughly
    # in issue order, so xa (gates L1) and xb go before everything else.
    xa = bigX.tile([128, QF], BF16, tag="slotA")      # xa -> (dead) -> h2
    nc.sync.dma_start(xa[:], d["xA4"][:])
    xb = bigY.tile([PH, C * 2 * B], BF16, tag="xb")
    xbv = xb[:].rearrange("p (c h a s) -> p c h a s", c=C, h=2, a=4, s=64)

    # ---------------- constants (sync ring; small)
    def load(name, shape, dt=F32, pool=sing):
        t = pool.tile(shape, dt, tag=name)
        nc.sync.dma_start(t[:], d[name][:])
        return t

    w1D = load("w1D", [128, 128], BF16)
    w2D = load("w2D", [128, 128], BF16)
    w3sp = load("w3sp", [128, 4], BF16)
    f1s = load("f1", [128, 16])
    ft1s = load("ft1", [16, 128])
    f2s = load("f2", [128, 8])
    ft2s = load("ft2", [8, 128])
    f8_16s = load("f8_16", [128, 16])
    f8_8s = load("f8_8", [64, 8])
    g1 = load("g1q", [128, 1]); be1 = load("be1q", [128, 1])
    g2 = load("g2q", [128, 1]); be2 = load("be2q", [128, 1])
    g3 = load("g3s", [1, 1]); be3 = load("be3s", [1, 1])
    fg1 = load("fg1s", [128, 1])
    fbe1 = load("fbe1s", [128, 1])
    fg2 = load("fg2t", [128, 2])
    fbe2 = load("fbe2t", [128, 2])
    fw2 = load("fw2t", [128, 256], BF16)
    ones128 = sing.tile([128, 1], F32)
    nc.vector.memset(ones128[:], 1.0)
    ones8 = sing.tile([8, 1], F32)
    nc.vector.memset(ones8[:], 1.0)
    ones125 = sing.tile([PH, 1], F32)
    nc.vector.memset(ones125[:], 1.0)
    ones1x = sing.tile([1, 128], F32)
    nc.vector.memset(ones1x[:], 1.0)
    ident = sing.tile([128, 128], F32)
    make_identity(nc, ident[:])
    ident16 = sing.tile([128, 128], BF16)
    make_identity(nc, ident16[:])

    # fw1 head: chunked 8-its per InstDMACopy on alternating rings, queued
    # behind xa/xb/consts.
    fwv = d["fw1t"][:].rearrange("p (i o) -> p i o", i=C * 2, o=1024)
    fwAt = fwA_p.tile([PH, FW_SPLIT, 1024], BF16, tag="fwA")
    for k in range(0, FW_SPLIT, 8):
        hi = min(k + 8, FW_SPLIT)
        nc.sync.dma_start(fwAt[:, k:hi, :], fwv[:, k:hi, :])
    # xb queues behind the fw1 head: it is only read by FC1 (~t+200us), and
    # keeping it off the front gives xa the full HBM bandwidth.
    nc.sync.dma_start(xb[:], d["xB"][:])


    def stage_layer(rhs_src, wD, fold, foldT, f8fold, st_i, st_o,
                    gq, beq, name, out_tag):
        """Quartered A-orientation layer: matmuls -> raw evict (scalar) +
        bn_stats (vector, from PSUM), fold + AllGather stats, then BN+relu
        applied in place, split scalar/vector."""
        y = bigX.tile([128, QF], BF16, tag=out_tag, name=f"y_{name}")
        stat = work.tile([128, 2 * NCHUNK, 6], F32, tag=f"stat_{name}")
        for j in range(NCHUNK):
            ps = psA.tile([128, 1024], F32, tag="psA", name=f"ps_{name}_{j}")
            base = j * NCH
            nc.tensor.matmul(ps[:, 0:500], wD[:], rhs_src[:, base:base + 500],
                             start=True, stop=True)
            nc.tensor.matmul(ps[:, 512:1012], wD[:],
                             rhs_src[:, base + 500:base + 1000],
                             start=True, stop=True)
            pv = ps[:].rearrange("p (k c) -> p k c", k=2, c=512)[:, :, 0:500]
            nc.scalar.copy(
                y[:, base:base + NCH].rearrange("p (k c) -> p k c", k=2,
                                                c=500), pv)
            nc.vector.bn_stats(stat[:, 2 * j, :], ps[:, 0:500])
            nc.vector.bn_stats(stat[:, 2 * j + 1, :], ps[:, 512:1012])
        mv = work.tile([128, 2], F32, tag=f"mv_{name}")
        nc.vector.bn_aggr(mv[:], stat[:])
        ss = _mkstats(nc, work, mv, QF, name)
        nfold = fold.shape[1]
        psf = psS.tile([128, 2], F32, tag="small", name=f"psf_{name}")
        nc.tensor.matmul(psf[:nfold, :], fold[:], ss[:], start=True, stop=True)
        sbf = work.tile([nfold, 2], F32, tag=f"sbf_{name}")
        nc.scalar.copy(sbf[:], psf[:nfold, :])
        if LOCAL_STATS:
            mr = _mv_from_ss(nc, work, sbf, B * P // NCORES, name)
        else:
            nc.gpsimd.dma_start(st_i[:], sbf[:])
            nc.gpsimd.collective_compute(
                "AllGather", ALU.bypass, replica_groups=RG,
                ins=[st_i[:]], outs=[st_o[:]])
            agg = work.tile([nfold * NCORES, 2], F32, tag=f"agg_{name}")
            nc.gpsimd.dma_start(agg[:], st_o[:])
            psg = psS.tile([128, 2], F32, tag="small", name=f"psg_{name}")
            nc.tensor.matmul(psg[:nfold, :], f8fold[:], agg[:], start=True,
                             stop=True)
            ssg = work.tile([nfold, 2], F32, tag=f"ssg_{name}")
            nc.scalar.copy(ssg[:], psg[:nfold, :])
            mr = _mv_from_ss(nc, work, ssg, B * P, name)
        psb = psS.tile([128, 2], F32, tag="small", name=f"psb_{name}")
        nc.tensor.matmul(psb[:], foldT[:], mr[:], start=True, stop=True)
        mrq = work.tile([128, 2], F32, tag=f"mrq_{name}")
        nc.scalar.copy(mrq[:], psb[:])
        sc, bi = _scale_bias(nc, work, mrq, gq, beq, name)
        # relu in place: scalar takes the first chunks, vector the rest
        NSC = 6
        for j in range(NSC):
            sl = slice(j * NCH, (j + 1) * NCH)
            nc.scalar.activation(y[:, sl], y[:, sl], AF.Relu,
                                 bias=bi[:], scale=sc[:])
        for j in range(NSC, NCHUNK):
            sl = slice(j * NCH, (j + 1) * NCH)
            nc.vector.tensor_scalar(y[:, sl], y[:, sl], sc[:], bi[:],
                                    ALU.mult, ALU.add)
            nc.vector.tensor_scalar_max(y[:, sl], y[:, sl], 0.0)
        return y

    # ---------------- stage A layers 1 & 2
    h1 = stage_layer(xa, w1D, f1s, ft1s, f8_16s,
                     d["st1_i"], d["st1_o"], g1, be1, "l1", "slotB")
    # h2 reuses slot A (xa dead after L1 matmuls)
    h2 = stage_layer(h1, w2D, f2s, ft2s, f8_8s,
                     d["st2_i"], d["st2_o"], g2, be2, "l2", "slotA")

    # ---------------- layer 3 via transpose-matmuls: scores point-major.
    # lhsT = h2[:, 125c:125c+125] (K=128 channel-partitions, M=125 points),
    # rhs = w3 spread [128, 4] (col a = w3 in quarter-a rows) ->
    # psL3[:, 4c+a] = score of quarter a's point 125c+p.
    # Free-dim layout: c = (s, h) with s in 0..63, h in 0..1; col = 8s+4h+a.
    psL3_cm = tc.tile_pool(name="psL3", bufs=1, space="PSUM")
    psL3 = psL3_cm.__enter__()
    l3ps = psL3.tile([PH, 512], F32, tag="l3ps")
    for cgrp in range(128):
        nc.tensor.matmul(l3ps[:, 4 * cgrp: 4 * cgrp + 4],
                         h2[:, 125 * cgrp: 125 * cgrp + 125],
                         w3sp[:], start=True, stop=True)
    # BN3 stats over all points (125*512 = 64000 local)
    stat3 = work.tile([PH, 6], F32, tag="stat3")
    nc.vector.bn_stats(stat3[:], l3ps[:])
    mv3 = work.tile([PH, 2], F32, tag="mv3")
    nc.vector.bn_aggr(mv3[:], stat3[:])
    ss3 = _mkstats(nc, work, mv3, 512, "l3")
    psf3 = psS.tile([128, 2], F32, tag="small", name="psf3")
    nc.tensor.matmul(psf3[:1, :], ones125[:], ss3[:], start=True, stop=True)
    sbf3 = work.tile([1, 2], F32, tag="sbf3")
    nc.scalar.copy(sbf3[:], psf3[:1, :])
    if LOCAL_STATS:
        mr3 = _mv_from_ss(nc, work, sbf3, B * P // NCORES, "l3")
    else:
        nc.gpsimd.dma_start(d["st3_i"][:], sbf3[:])
        nc.gpsimd.collective_compute(
            "AllGather", ALU.bypass, replica_groups=RG,
            ins=[d["st3_i"][:]], outs=[d["st3_o"][:]])
        agg3 = work.tile([8, 2], F32, tag="agg3")
        nc.gpsimd.dma_start(agg3[:], d["st3_o"][:])
        psg3 = psS.tile([128, 2], F32, tag="small", name="psg3")
        nc.tensor.matmul(psg3[:1, :], ones8[:], agg3[:], start=True, stop=True)
        ssg3 = work.tile([1, 2], F32, tag="ssg3")
        nc.scalar.copy(ssg3[:], psg3[:1, :])
        mr3 = _mv_from_ss(nc, work, ssg3, B * P, "l3")
    scb1 = work.tile([1, 2], F32, tag="scb1")
    nc.vector.tensor_mul(scb1[:, 0:1], g3[:], mr3[:, 1:2])
    nc.vector.tensor_mul(scb1[:, 1:2], scb1[:, 0:1], mr3[:, 0:1])
    nc.vector.tensor_sub(scb1[:, 1:2], be3[:], scb1[:, 1:2])
    psb3 = psS.tile([128, 2], F32, tag="small", name="psb3")
    nc.tensor.matmul(psb3[:PH, :], ones1x[:, :PH], scb1[:], start=True,
                     stop=True)
    scb = work.tile([PH, 2], F32, tag="scb")
    nc.scalar.copy(scb[:], psb3[:PH, :])
    # relu(BN3) in place on PSUM, then exp -> attention numerators
    nc.scalar.activation(l3ps[:], l3ps[:], AF.Relu,
                         bias=scb[:, 1:2], scale=scb[:, 0:1])
    expP = bigY.tile([PH, 512], BF16, tag="expP")
    nc.scalar.activation(expP[:], l3ps[:], AF.Exp)
    expv = expP[:].rearrange("p (s h a) -> p h a s", s=64, h=2, a=4)
    # partial softmax denominators: sum over h (vector) then partitions (PE)
    zpart = work.tile([PH, 256], F32, tag="zpart")
    zpv = zpart[:].rearrange("p (a s) -> p a s", a=4, s=64)
    nc.vector.tensor_add(zpv, expv[:, 0], expv[:, 1])
    psz = psS.tile([128, 256], F32, tag="psz")
    nc.tensor.matmul(psz[:1, :], ones125[:], zpart[:], start=True, stop=True)
    zq = work.tile([1, 256], BF16, tag="zq")
    nc.scalar.copy(zq[:], psz[:1, :])
    # z into every shard's aux row of rs5_i (cols already in segment order)
    for cc in range(NCORES):
        nc.sync.dma_start(d["rs5_i"][cc * 129 + 128: cc * 129 + 129, :],
                          zq[:])

    psL3_cm.__exit__(None, None, None)
    psS_cm.__exit__(None, None, None)
    psA_cm.__exit__(None, None, None)
    bigX_cm.__exit__(None, None, None)

    # xa/h1/h2 are dead now -> their SBUF region hosts the fw1 tail chunk
    # and the small tail tiles.
    big2_cm = tc.tile_pool(name="big2", bufs=1)
    big2 = big2_cm.__enter__()
    fwBt = big2.tile([PH, C * 2 - FW_SPLIT, 1024], BF16, tag="fwB")
    for k in range(0, C * 2 - FW_SPLIT, 8):
        hi = min(k + 8, C * 2 - FW_SPLIT)
        nc.sync.dma_start(fwBt[:, k:hi, :],
                          fwv[:, FW_SPLIT + k:FW_SPLIT + hi, :])

    # ---------------- FC1 (contraction-sharded, out [1024, 256] partial)
    psF_cm = tc.tile_pool(name="psF", bufs=1, space="PSUM")
    ptp_cm = tc.tile_pool(name="ptp", bufs=3)
    psF = psF_cm.__enter__()
    ptp = ptp_cm.__enter__()
    # pt-stationary "swap" arrangement: lhsT = a 128-segment half of pt,
    # rhs = the full 1024-wide fw row (fp16 moving operand) -> out is
    # [seg-half, fc1out] in PSUM; 2 matmuls per it instead of 8 (each
    # InstMatmult carries a fused LDWEIGHTS here, so fewer/wider wins).
    r1ps = [psF.tile([128, 1024], F32, name=f"r1ps_{hh}", tag=f"r1_{hh}")
            for hh in range(2)]
    NIT = C * 2
    for ch in range(C):
        for h in range(2):
            it = ch * 2 + h
            fw = (fwAt[:, it, :] if it < FW_SPLIT
                  else fwBt[:, it - FW_SPLIT, :])
            pt = ptp.tile([PH, 256], BF16, tag="pt", name=f"pt_{it}")
            ptv = pt[:].rearrange("p (a s) -> p a s", a=4, s=64)
            nc.vector.tensor_mul(ptv, xbv[:, ch, h], expv[:, h])
            for hh in range(2):
                for q in range(2):
                    nc.tensor.matmul(
                        r1ps[hh][:, q * 512: q * 512 + 512],
                        pt[:, hh * 128: hh * 128 + 128],
                        fw[:, q * 512: q * 512 + 512],
                        start=(it == 0), stop=(it == NIT - 1))
    # transpose [seg, out] -> [out, seg] before staging (RS shards are
    # out-major).  Evict each half to fp16, then 16 PE transposes.
    # per-block evict -> PE transpose -> stage, pipelined across scalar /
    # vector / tensor; everything fits in psF (r1ps 4 banks + fc1T 2).
    _rings = [nc.sync, nc.sync]
    r1fl = [None, None]
    for hh in range(2):
        r1fl[hh] = big2.tile([128, 1024], BF16, tag=f"r1fl_{hh}",
                             name=f"r1fl_{hh}")
    for m in range(8):
        for hh in range(2):
            blk = slice(m * 128, m * 128 + 128)
            if (m + hh) % 2 == 0:
                nc.scalar.copy(r1fl[hh][:, blk], r1ps[hh][:, blk])
            else:
                nc.vector.tensor_copy(r1fl[hh][:, blk], r1ps[hh][:, blk])
    for m in range(8):
        r1sb = big2.tile([128, 256], BF16, tag="r1sb", name=f"r1sb_{m}",
                         bufs=2)
        for hh in range(2):
            ps_t = psF.tile([128, 128], BF16, tag="fc1T",
                            name=f"fc1T_{m}_{hh}", bufs=2)
            nc.tensor.transpose(ps_t[:], r1fl[hh][:, m * 128: m * 128 + 128],
                                ident16[:])
            nc.scalar.copy(r1sb[:, hh * 128: hh * 128 + 128], ps_t[:])
        _rings[m % 2].dma_start(d["rs5_i"][m * 129: m * 129 + 128, :],
                                r1sb[:])
    nc.gpsimd.collective_compute(
        "ReduceScatter", ALU.add, replica_groups=RG,
        ins=[d["rs5_i"][:]], outs=[d["rs5_o"][:]])

    ptp_cm.__exit__(None, None, None)
    psF_cm.__exit__(None, None, None)

    # ---------------- FC1 finish + FC2 + tail
    ps2_cm = tc.tile_pool(name="ps2", bufs=1, space="PSUM")
    ps2 = ps2_cm.__enter__()

    r1h = big2.tile([128, 256], BF16, tag="r1h")
    nc.sync.dma_start(r1h[:], d["rs5_o"][0:128, :])
    zrow = work.tile([1, 256], BF16, tag="zrow")
    nc.scalar.dma_start(zrow[:], d["rs5_o"][128:129, :])
    zrec = work.tile([1, 256], F32, tag="zrec")
    nc.vector.reciprocal(zrec[:], zrow[:])
    ps_z = ps2.tile([128, 256], F32, tag="nrmb", name="zb")
    nc.tensor.matmul(ps_z[:], ones1x[:], zrec[:], start=True, stop=True)
    zinv = big2.tile([128, 256], BF16, tag="zinv")
    nc.scalar.copy(zinv[:], ps_z[:])
    r1 = big2.tile([128, 256], F32, tag="r1")
    nc.vector.tensor_mul(r1[:], r1h[:], zinv[:])
    # BN over segments (free dim), relu
    stf1 = work.tile([128, 6], F32, tag="stf1")
    nc.vector.bn_stats(stf1[:], r1[:])
    mvf1 = work.tile([128, 2], F32, tag="mvf1")
    nc.vector.bn_aggr(mvf1[:], stf1[:])
    epsf = work.tile([128, 1], F32, tag="epsf")
    nc.vector.memset(epsf[:], EPS_BN)
    nc.scalar.activation(mvf1[:, 1:2], mvf1[:, 1:2], AF.Sqrt, bias=epsf[:])
    nc.vector.reciprocal(mvf1[:, 1:2], mvf1[:, 1:2])
    scf1, bif1 = _scale_bias(nc, work, mvf1, fg1, fbe1, "f1")
    r1b = big2.tile([128, 256], BF16, tag="r1b")
    nc.scalar.activation(r1b[:], r1[:], AF.Relu, bias=bif1[:], scale=scf1[:])
    # FC2 partial
    r2sb = big2.tile([128, 2, 256], BF16, tag="r2sb")
    for m in range(2):
        ps_r2 = ps2.tile([128, 256], F32, tag=f"r2_{m}")
        nc.tensor.matmul(ps_r2[:], fw2[:, m * 128: (m + 1) * 128], r1b[:],
                         start=True, stop=True)
        nc.scalar.copy(r2sb[:, m, :], ps_r2[:])
        _rings[m].dma_start(d["ar6_i"][m * 128: (m + 1) * 128, :],
                            r2sb[:, m, :])
    nc.gpsimd.collective_compute(
        "AllReduce", ALU.add, replica_groups=RG,
        ins=[d["ar6_i"][:]], outs=[d["ar6_o"][:]])

    # tail: BN over segments per o2-row + relu (both blocks), L2-norm via
    # ones-matmul row-sums of squares (before the transposes), then transpose.
    r2s = []
    ps_n = ps2.tile([128, 256], F32, tag="nrm")
    for m in range(2):
        r2 = big2.tile([128, 256], BF16, tag=f"r2_{m}", name=f"r2_{m}")
        _rings[m].dma_start(r2[:], d["ar6_o"][m * 128: (m + 1) * 128, :])
        stf2 = work.tile([128, 6], F32, tag=f"stf2_{m}")
        nc.vector.bn_stats(stf2[:], r2[:])
        mvf2 = work.tile([128, 2], F32, tag=f"mvf2_{m}")
        nc.vector.bn_aggr(mvf2[:], stf2[:])
        nc.scalar.activation(mvf2[:, 1:2], mvf2[:, 1:2], AF.Sqrt, bias=epsf[:])
        nc.vector.reciprocal(mvf2[:, 1:2], mvf2[:, 1:2])
        scf2, bif2 = _scale_bias(nc, work, mvf2,
                                 fg2[:, m: m + 1], fbe2[:, m: m + 1],
                                 f"f2_{m}")
        nc.scalar.activation(r2[:], r2[:], AF.Relu, bias=bif2[:], scale=scf2[:])
        sq = big2.tile([128, 256], F32, tag=f"sq_{m}", name=f"sq_{m}")
        nc.scalar.activation(sq[:], r2[:], AF.Square)
        nc.tensor.matmul(ps_n[:1, :], ones128[:], sq[:],
                         start=(m == 0), stop=(m == 1))
        r2s.append(r2)
    nrm = work.tile([1, 256], F32, tag="nrmrow")
    nc.scalar.activation(nrm[:], ps_n[:1, :], AF.Sqrt)
    nc.vector.tensor_scalar_max(nrm[:], nrm[:], 1e-12)
    nc.vector.reciprocal(nrm[:], nrm[:])
    ps_nb = ps2.tile([128, 256], F32, tag="nrmb")
    nc.tensor.matmul(ps_nb[:], ones1x[:], nrm[:], start=True, stop=True)
    nrmb = big2.tile([128, 256], BF16, tag="nrmbs")
    nc.scalar.copy(nrmb[:], ps_nb[:])
    outT = big2.tile([128, 2, 256], F32, tag="outT")
    for m in range(2):
        nc.vector.tensor_mul(r2s[m][:], r2s[m][:], nrmb[:])
        for tt in range(2):
            ps_t = ps2.tile([128, 128], BF16, tag="tailT",
                            name=f"tailT_{m}_{tt}", bufs=2)
            nc.tensor.transpose(ps_t[:], r2s[m][:, tt * 128: (tt + 1) * 128],
                                ident16[:])
            nc.scalar.copy(outT[:, tt, m * 128: (m + 1) * 128], ps_t[:])
    for tt in range(2):
        _rings[tt].dma_start(d["out_final"][tt * 128: (tt + 1) * 128, :],
                             outT[:, tt, :])

    ps2_cm.__exit__(None, None, None)
    big2_cm.__exit__(None, None, None)
    fwA_cm.__exit__(None, None, None)
    work_cm.__exit__(None, None, None)
    bigY_cm.__exit__(None, None, None)
    sing_cm.__exit__(None, None, None)


# ------------------------------------------------------------------ host side
def _prep_core(x3, fw1, c):
    xs = x3[:, PL * c: PL * (c + 1), :]                        # [256,250,32]
    arr = np.ascontiguousarray(xs.transpose(2, 0, 1))          # [32,256,250]
    xA4 = arr.reshape(C, 4, QF).transpose(1, 0, 2).reshape(128, QF)
    xb = xs.reshape(B, 2, PH, C).transpose(2, 3, 1, 0)         # [125,32,2,256]
    xB = np.ascontiguousarray(xb).reshape(PH, C * 2 * B)
    fw = fw1.reshape(1024, P, C)[:, PL * c: PL * (c + 1), :]
    fw = fw.reshape(1024, 2, PH, C).transpose(2, 3, 1, 0)      # [125,32,2,1024]
    fw1t = np.ascontiguousarray(fw).reshape(PH, C * 2 * 1024)
    bf = np.float16
    return (np.ascontiguousarray(xA4).astype(bf), xB.astype(bf),
            fw1t.astype(bf))


def _qrep(v, rows):
    out = np.zeros((128, 1), np.float32)
    for a in range(4):
        out[32 * a: 32 * a + rows, 0] = v
    return out


def _wdiag(w):
    """w [out,in] -> block-diagonal lhsT [128, 128]: block a (32x32) holds
    w.T in its top-left corner."""
    t = np.zeros((128, 128), np.float32)
    wt = w.T  # [in, out]
    for a in range(4):
        t[32 * a: 32 * a + wt.shape[0], 32 * a: 32 * a + wt.shape[1]] = wt
    return t


def kernel(**inputs):
    if "nc" not in _cache:
        _cache["nc"] = _build()
    nc = _cache["nc"]
    bf = np.float16

    g = {k: np.asarray(v, np.float32) for k, v in inputs.items()
         if k != "length"}
    x3 = g["x"].reshape(B, P, C)

    f1 = np.zeros((128, 16), np.float32)
    f2 = np.zeros((128, 8), np.float32)
    for a in range(4):
        f1[32 * a: 32 * a + 16, :] = np.eye(16, dtype=np.float32)
        f2[32 * a: 32 * a + 8, :] = np.eye(8, dtype=np.float32)
    f8_16 = np.zeros((128, 16), np.float32)
    f8_8 = np.zeros((64, 8), np.float32)
    for k in range(8):
        f8_16[16 * k: 16 * k + 16, :] = np.eye(16, dtype=np.float32)
        f8_8[8 * k: 8 * k + 8, :] = np.eye(8, dtype=np.float32)
    w3sp = np.zeros((128, 4), np.float32)
    for a in range(4):
        w3sp[32 * a: 32 * a + 8, a] = g["w3"][0, :]

    shared = {
        "w1D": _wdiag(g["w1"]).astype(bf),
        "w2D": _wdiag(g["w2"]).astype(bf),
        "w3sp": w3sp.astype(bf),
        "g1q": _qrep(g["g1"], 16), "be1q": _qrep(g["be1"], 16),
        "g2q": _qrep(g["g2"], 8), "be2q": _qrep(g["be2"], 8),
        "g3s": g["g3"].reshape(1, 1), "be3s": g["be3"].reshape(1, 1),
        "f1": f1, "ft1": np.ascontiguousarray(f1.T),
        "f2": f2, "ft2": np.ascontiguousarray(f2.T),
        "f8_16": f8_16, "f8_8": f8_8,
        "fg2t": np.ascontiguousarray(g["fg2"].reshape(2, 128).T),
        "fbe2t": np.ascontiguousarray(g["fbe2"].reshape(2, 128).T),
    }

    in_maps = []
    for c in range(NCORES):
        xA4, xB, fw1t = _prep_core(x3, g["fw1"], c)
        m = dict(shared)
        m["xA4"] = xA4
        m["xB"] = xB
        m["fw1t"] = fw1t
        m["fw2t"] = np.ascontiguousarray(
            g["fw2"][:, 128 * c: 128 * (c + 1)].T).astype(bf)
        m["fg1s"] = g["fg1"][128 * c: 128 * (c + 1)].reshape(128, 1)
        m["fbe1s"] = g["fbe1"][128 * c: 128 * (c + 1)].reshape(128, 1)
        in_maps.append(m)

    from concourse.bass_utils import run_bass_kernel_spmd

    res = run_bass_kernel_spmd(nc, in_maps, core_ids=list(range(NCORES)),
                               trace=bool(_cache.get("trace")))
    _cache["last_result"] = res
    return np.asarray(res.results[0]["out_final"], np.float32)


if __name__ == "__main__":
    nc = _build()
    print("build ok; instructions:",
          sum(len(bb.instructions) for bb in nc.main_func.blocks))
